# revision 1
# baseline (speedup 1.0000x reference)
"""Trainium2 Bass kernel for nn_BuildK (27-neighborhood kNN softmax weights).

Strategy: shard the y dimension across 8 NeuronCores (spatial parallel, no
cross-core communication). Each core receives a halo-extended, x-rotated input
slab, computes per-voxel: the 9 intensity-nearest of its 27 periodic neighbors
(stable selection network on f32 |diff| keys with an fp16 index payload),
reconstructs the sorted neighbor values exactly via sign*distance, forms the
pairwise feature-row distances through 27 shifted dot planes in f32, and
applies a rowwise softmax. Output is gathered and reassembled on the host.
"""

import sys

sys.path.insert(0, "/opt/trn_rl_repo")

import numpy as np

H, M, N = 64, 128, 128
NCORES = 8
YS = M // NCORES          # 16 owned y rows per core
YE = YS + 2               # 18 = sort region (owned + 1 halo each side)
YI = YS + 4               # 20 = input slab y extent (halo 2)
ZE = H + 2                # 66 = z extent with periodic wrap rows
KN = 9
EPS = 1e-6

POOL_CE_EVERY = 1000000   # min/max stay on DVE (Pool lacks min/max opcodes)
POOL_RANKS = ()           # apply-chain ranks owned by gpsimd
ACT_OCC = False           # build occ masks on ScalarE (2-op trick) for Pool ranks
DOT_DVE_EVERY = 3         # 1-in-n dot multiplies on DVE, rest on gpsimd
BLEND_POOL_EVERY = 2      # 1-in-n comparator blend groups on gpsimd
POOL_DOT_EVERY = 3        # every n'th dot multiply goes to gpsimd
NSLOT = 36                # wire slots for the selection network


# --------------------------------------------------------------------------
# Selection network: top-9-sorted of 27, built from three 9-sorters and two
# pruned odd-even merges.  Ops are liveness-annotated for outputs 1..8
# (output 0 is always the center voxel: d=0, w=c).
# --------------------------------------------------------------------------

_SORT9 = [(0, 3), (1, 7), (2, 5), (4, 8), (0, 7), (2, 4), (3, 8), (5, 6),
          (0, 2), (1, 3), (4, 5), (7, 8), (1, 4), (3, 6), (5, 7), (0, 1),
          (2, 4), (3, 5), (6, 8), (2, 3), (4, 5), (6, 7), (1, 2), (3, 4),
          (5, 6)]


def _oddeven_merge(lo, n, r, out):
    step = r * 2
    if step < n:
        _oddeven_merge(lo, n, step, out)
        _oddeven_merge(lo + r, n, step, out)
        for i in range(lo + r, lo + n - r, step):
            out.append((i, i + r))
    else:
        out.append((lo, lo + r))


def _merge_topk(lenA, lenB, k):
    ces = []
    _oddeven_merge(0, 32, 1, ces)
    inf = [False] * 32
    for w in range(lenA, 16):
        inf[w] = True
    for w in range(16 + lenB, 32):
        inf[w] = True
    label = list(range(32))
    kept = []
    for (i, j) in ces:
        if inf[i] and inf[j]:
            continue
        if inf[j] and not inf[i]:
            continue
        if inf[i] and not inf[j]:
            label[i], label[j] = label[j], label[i]
            inf[i], inf[j] = False, True
            continue
        kept.append((label[i], label[j]))
    needed = set(label[w] for w in range(k))
    keep = []
    for (i, j) in reversed(kept):
        if i in needed or j in needed:
            keep.append((i, j))
            needed.add(i)
            needed.add(j)
    keep.reverse()

    def rm(w):
        return w if w < 16 else w - 16 + lenA

    return [(rm(i), rm(j)) for (i, j) in keep], [rm(label[w]) for w in range(k)]


def build_network():
    """Top-8-sorted of the 26 non-center candidates.  Wires 0..25 map to
    candidates CAND[w] (reference enumeration minus the center 13).
    Returns (ops, outw, cand): ops = [(i, j, need_i, need_j)]."""
    cand = [d for d in range(27) if d != 13]
    S8 = [(0, 1), (2, 3), (4, 5), (6, 7), (0, 2), (1, 3), (4, 6), (5, 7),
          (1, 2), (5, 6), (0, 4), (3, 7), (1, 5), (2, 6), (1, 4), (3, 6),
          (2, 4), (3, 5), (3, 4)]
    net = []
    net += [(i, j) for (i, j) in _SORT9]
    net += [(i + 9, j + 9) for (i, j) in _SORT9]
    net += [(i + 18, j + 18) for (i, j) in S8]
    m1, ow1 = _merge_topk(9, 9, 8)
    net += m1
    m2, ow2 = _merge_topk(8, 8, 8)
    remap = {i: ow1[i] for i in range(8)}
    remap.update({8 + i: 18 + i for i in range(8)})
    net += [(remap[i], remap[j]) for (i, j) in m2]
    outw = [remap[w] for w in ow2]

    live = set(outw)
    ops = []
    for (i, j) in reversed(net):
        ni, nj = i in live, j in live
        if not (ni or nj):
            continue
        ops.append((i, j, ni, nj))
        live.add(i)
        live.add(j)
    ops.reverse()
    return ops, outw, cand


NET_OPS, NET_OUTW, CAND = build_network()

OFFS = [(oz, oy, ox) for oz in (-1, 0, 1) for oy in (-1, 0, 1)
        for ox in (-1, 0, 1)]            # reference enumeration; 13 = center


# --------------------------------------------------------------------------
# Bass graph
# --------------------------------------------------------------------------

def build_bass(ks_value: float, reps: int = 1):
    from concourse import bacc, mybir
    from concourse import tile
    from concourse.alu_op_type import AluOpType as op

    f32 = mybir.dt.float32
    f16 = mybir.dt.float16
    AF = mybir.ActivationFunctionType

    nc = bacc.Bacc("TRN2", target_bir_lowering=False, debug=False,
                   num_devices=NCORES)

    xin = nc.dram_tensor("xin", [128, 3, ZE, YI], f32, kind="ExternalInput").ap()
    outd = nc.dram_tensor("out", [128, H, YS, KN], f32,
                          kind="ExternalOutput").ap()

    ZC = 16                      # z chunk for both phases
    FS = ZC * YE                 # 288 free elems in sort phase

    dve = nc.vector
    act = nc.scalar
    gp = nc.gpsimd

    ce_ctr = [0]

    def minmax_engine():
        ce_ctr[0] += 1
        return gp if (ce_ctr[0] % POOL_CE_EVERY == 0) else dve

    with tile.TileContext(nc) as tc:
      for _rep in range(reps):
        with tc.tile_pool(name="pp", bufs=1) as pp:
            Wslab = pp.tile([128, KN, ZE, YE], f32, tag="Wslab")
            idx9 = pp.tile([128, KN, H, YE], f16, tag="idx9")

            # ---------------- sort phase ----------------
            with tc.tile_pool(name="xp", bufs=1) as xp, \
                 tc.tile_pool(name="sortp", bufs=1) as sp:
                X3 = xp.tile([128, 3, ZE, YI], f32, tag="X3")
                nc.sync.dma_start(out=X3[:], in_=xin[:])
                kbig = sp.tile([128, NSLOT, FS], f32, tag="kbig")

                for zc in range(0, H, ZC):
                    cvw = X3[:, 1, 1 + zc:1 + zc + ZC, 1:1 + YE]

                    def vview(d):
                        oz, oy, ox = OFFS[d]
                        return X3[:, ox + 1,
                                  1 + zc + oz:1 + zc + oz + ZC,
                                  1 + oy:1 + oy + YE]

                    free_slots = list(range(NSLOT))
                    wire_slot = {}

                    def k_ap(s):
                        return kbig[:, s, :]

                    with tc.tile_pool(name="chunkp", bufs=1) as cp:
                        stmp = [cp.tile([128, FS], f32, name=f"s{i}", tag=f"s{i}")
                                for i in range(4)]
                        ptmp = [cp.tile([128, FS], f32, name=f"pt{i}",
                                        tag=f"pt{i}") for i in range(12)]
                        for w, d in enumerate(CAND):
                            s = free_slots.pop()
                            wire_slot[w] = s
                            st = stmp[d % 4]
                            eng = gp if d % 2 else dve
                            eng.tensor_tensor(out=st[:], in0=vview(d),
                                              in1=cvw, op=op.subtract)
                            dd = ptmp[(3 * d) % 12]
                            c1 = ptmp[(3 * d + 1) % 12]
                            bb = ptmp[(3 * d + 2) % 12]
                            dve.scalar_tensor_tensor(
                                out=dd[:], in0=st[:], scalar=-1.0,
                                in1=st[:], op0=op.mult, op1=op.max)
                            dve.tensor_scalar(out=c1[:], in0=dd[:],
                                              scalar1=257.0, scalar2=None,
                                              op0=op.mult)
                            veng = gp if d % 2 else dve
                            veng.tensor_tensor(out=dd[:], in0=c1[:],
                                               in1=dd[:], op=op.subtract)
                            veng.tensor_tensor(out=c1[:], in0=c1[:],
                                               in1=dd[:], op=op.subtract)
                            # c1 = hi; multiplier m = 1 + (2d+b)*2^-23
                            dve.tensor_scalar(out=bb[:], in0=st[:],
                                              scalar1=0.0, scalar2=None,
                                              op0=op.is_gt)
                            dve.tensor_scalar(
                                out=bb[:], in0=bb[:],
                                scalar1=float(np.float32(2.0**-23)),
                                scalar2=float(np.float32(1.0 + 2 * d * 2.0**-23)),
                                op0=op.mult, op1=op.add)
                            veng.tensor_tensor(out=k_ap(s), in0=c1[:],
                                               in1=bb[:], op=op.mult)

                        for n, (i, j, ni, nj) in enumerate(NET_OPS):
                            si, sj = wire_slot[i], wire_slot[j]
                            new_i = free_slots.pop() if ni else None
                            new_j = free_slots.pop() if nj else None
                            eng = minmax_engine()
                            if ni:
                                eng.tensor_tensor(out=k_ap(new_i),
                                                  in0=k_ap(si), in1=k_ap(sj),
                                                  op=op.min)
                            if nj:
                                eng.tensor_tensor(out=k_ap(new_j),
                                                  in0=k_ap(si), in1=k_ap(sj),
                                                  op=op.max)
                            free_slots.append(si)
                            free_slots.append(sj)
                            if ni:
                                wire_slot[i] = new_i
                            else:
                                del wire_slot[i]
                            if nj:
                                wire_slot[j] = new_j
                            else:
                                del wire_slot[j]

                        # unpack sorted keys: hi (distance), delta, sign;
                        # reconstruct w_r = c + sign*hi directly
                        dsor = [cp.tile([128, FS], f32, name=f"ds{r}",
                                        tag=f"ds{r}") for r in range(8)]
                        ROUND_C = float(1.5 * 2.0**23)
                        for r in range(1, KN):
                            pk = k_ap(wire_slot[NET_OUTW[r - 1]])
                            e1 = ptmp[(3 * r) % 12]
                            e2 = ptmp[(3 * r + 1) % 12]
                            e3 = ptmp[(3 * r + 2) % 12]
                            hi = dsor[r - 1]
                            ueng = gp if r % 2 == 0 else dve
                            dve.tensor_scalar(out=e1[:], in0=pk,
                                              scalar1=257.0, scalar2=None,
                                              op0=op.mult)
                            ueng.tensor_tensor(out=e2[:], in0=e1[:], in1=pk,
                                               op=op.subtract)
                            ueng.tensor_tensor(out=hi[:], in0=e1[:],
                                               in1=e2[:], op=op.subtract)
                            ueng.tensor_tensor(out=e1[:], in0=pk, in1=hi[:],
                                               op=op.subtract)
                            dve.tensor_scalar(out=e3[:], in0=hi[:],
                                              scalar1=1e-30, scalar2=None,
                                              op0=op.add)
                            dve.reciprocal(out=e2[:], in_=e3[:])
                            dve.tensor_tensor(out=e1[:], in0=e1[:],
                                              in1=e2[:], op=op.mult)
                            dve.tensor_scalar(out=e1[:], in0=e1[:],
                                              scalar1=float(2.0**23),
                                              scalar2=ROUND_C, op0=op.mult,
                                              op1=op.add)
                            # e1 = code + ROUND_C ; code = 2*delta + signbit
                            dve.tensor_scalar(out=e2[:], in0=e1[:],
                                              scalar1=ROUND_C, scalar2=None,
                                              op0=op.subtract)
                            # delta = round((code - 0.5) * 0.5)
                            dve.tensor_scalar(out=e3[:], in0=e2[:],
                                              scalar1=-0.5, scalar2=0.5,
                                              op0=op.add, op1=op.mult)
                            dve.tensor_scalar(out=e3[:], in0=e3[:],
                                              scalar1=ROUND_C, scalar2=None,
                                              op0=op.add)
                            dve.tensor_scalar(out=e3[:], in0=e3[:],
                                              scalar1=ROUND_C, scalar2=None,
                                              op0=op.subtract)
                            act.activation(out=idx9[:, r, zc:zc + ZC, :],
                                           in_=e3[:], func=AF.Copy)
                            # sign = (code - 2*delta)*2 - 1
                            dve.scalar_tensor_tensor(out=e1[:], in0=e3[:],
                                                     scalar=-2.0, in1=e2[:],
                                                     op0=op.mult, op1=op.add)
                            dve.tensor_scalar(out=e1[:], in0=e1[:],
                                              scalar1=2.0, scalar2=-1.0,
                                              op0=op.mult, op1=op.add)
                            # w_r = c + sign*hi
                            ueng.tensor_tensor(out=e2[:], in0=e1[:],
                                               in1=hi[:], op=op.mult)
                            ueng.tensor_tensor(
                                out=Wslab[:, r, 1 + zc:1 + zc + ZC, :],
                                in0=e2[:], in1=cvw, op=op.add)

                        act.activation(out=Wslab[:, 0, 1 + zc:1 + zc + ZC, :],
                                       in_=cvw, func=AF.Copy)

            # ---------------- z wrap rows of Wslab ----------------
            nc.sync.dma_start(out=Wslab[:, :, 0:1, :],
                              in_=Wslab[:, :, H:H + 1, :])
            nc.sync.dma_start(out=Wslab[:, :, ZE - 1:ZE, :],
                              in_=Wslab[:, :, 1:2, :])

            # ---------------- sigma / scale planes ----------------
            with tc.tile_pool(name="spp", bufs=1) as spp:
                B3 = spp.tile([128, 3, ZE, YE], f32, tag="B3")
                scalem = spp.tile([128, H, YS], f32, tag="scalem")
                Cp = spp.tile([128, H, YS], f32, tag="Cp")

                with tc.tile_pool(name="sigt", bufs=1) as sg2:
                    S1 = sg2.tile([128, ZE, YE], f32, tag="S1")
                    S2 = sg2.tile([128, ZE, YE], f32, tag="S2")
                    sq = sg2.tile([128, ZE, YE], f32, tag="sq")
                    dve.tensor_tensor(out=S1[:], in0=Wslab[:, 0],
                                      in1=Wslab[:, 1], op=op.add)
                    for r in range(2, KN):
                        gp.tensor_tensor(out=S1[:], in0=S1[:],
                                         in1=Wslab[:, r], op=op.add)
                    act.activation(out=S2[:], in_=Wslab[:, 0], func=AF.Square)
                    for r in range(1, KN):
                        act.activation(out=sq[:], in_=Wslab[:, r],
                                       func=AF.Square)
                        gp.tensor_tensor(out=S2[:], in0=S2[:], in1=sq[:],
                                         op=op.add)
                    dve.scalar_tensor_tensor(out=B3[:, 1], in0=S1[:],
                                             scalar=-2.0 * EPS, in1=S2[:],
                                             op0=op.mult, op1=op.add)
                    nc.sync.dma_start(out=B3[:, 0][1:128], in_=B3[:, 1][0:127])
                    nc.sync.dma_start(out=B3[:, 0][0:1], in_=B3[:, 1][127:128])
                    nc.sync.dma_start(out=B3[:, 2][0:127], in_=B3[:, 1][1:128])
                    nc.sync.dma_start(out=B3[:, 2][127:128], in_=B3[:, 1][0:1])

                    S1o = S1[:, 1:1 + H, 1:1 + YS]
                    S2o = S2[:, 1:1 + H, 1:1 + YS]
                    sq1 = sg2.tile([128, H, YS], f32, tag="sq1")
                    tvar = sg2.tile([128, H, YS], f32, tag="tvar")
                    tmpv = sg2.tile([128, H, YS], f32, tag="tmpv")
                    rec = sg2.tile([128, H, YS], f32, tag="rec")
                    act.activation(out=sq1[:], in_=S1o, func=AF.Square)
                    dve.scalar_tensor_tensor(out=tvar[:], in0=sq1[:],
                                             scalar=-1.0 / 9.0, in1=S2o,
                                             op0=op.mult, op1=op.add)
                    dve.tensor_scalar(out=tmpv[:], in0=tvar[:], scalar1=0.0,
                                      scalar2=None, op0=op.is_equal)
                    dve.tensor_tensor(out=tmpv[:], in0=tmpv[:], in1=tvar[:],
                                      op=op.add)
                    dve.reciprocal(out=rec[:], in_=tmpv[:])
                    dve.tensor_scalar(out=rec[:], in0=rec[:],
                                      scalar1=-4.0 / (ks_value * ks_value),
                                      scalar2=None, op0=op.mult)
                    dve.tensor_scalar(out=tmpv[:], in0=tvar[:], scalar1=0.0,
                                      scalar2=None, op0=op.not_equal)
                    dve.tensor_tensor(out=scalem[:], in0=rec[:], in1=tmpv[:],
                                      op=op.mult)
                    dve.scalar_tensor_tensor(out=Cp[:], in0=S1o,
                                             scalar=2.0 * EPS, in1=S2o,
                                             op0=op.mult, op1=op.add)
                    dve.tensor_scalar(out=Cp[:], in0=Cp[:],
                                      scalar1=9.0 * EPS * EPS, scalar2=None,
                                      op0=op.add)

                # ---------------- dots + softmax phase ----------------
                with tc.tile_pool(name="dotp", bufs=1) as dp:
                    for zc in range(0, H, ZC):
                        wr0 = dp.tile([128, KN, ZC + 2, YE], f32, tag="wr0")
                        wr2 = dp.tile([128, KN, ZC + 2, YE], f32, tag="wr2")
                        src = Wslab[:, :, zc:zc + ZC + 2, :]
                        nc.sync.dma_start(out=wr0[1:128], in_=src[0:127])
                        nc.sync.dma_start(out=wr0[0:1], in_=src[127:128])
                        nc.sync.dma_start(out=wr2[0:127], in_=src[1:128])
                        nc.sync.dma_start(out=wr2[127:128], in_=src[0:1])

                        est = dp.tile([128, 27, ZC, YS], f16, tag="est")
                        NR = 3
                        prodTs = [dp.tile([128, ZC * YS, KN], f32,
                                          name=f"prodT{i}", tag=f"prodT{i}")
                                  for i in range(NR)]
                        dreds = [dp.tile([128, ZC, YS], f32,
                                         name=f"dred{i}", tag=f"dred{i}")
                                 for i in range(NR)]
                        t1s = [dp.tile([128, ZC, YS], f32,
                                       name=f"t1_{i}", tag=f"t1_{i}")
                               for i in range(NR)]
                        t2s = [dp.tile([128, ZC, YS], f32,
                                       name=f"t2_{i}", tag=f"t2_{i}")
                               for i in range(NR)]
                        scv = scalem[:, zc:zc + ZC, :]
                        cpv = Cp[:, zc:zc + ZC, :]

                        wA = Wslab[:, :, 1 + zc:1 + zc + ZC, 1:1 + YS]
                        for d in range(27):
                            oz, oy, ox = OFFS[d]
                            if ox == 0:
                                wB = Wslab[:, :,
                                           1 + zc + oz:1 + zc + oz + ZC,
                                           1 + oy:1 + oy + YS]
                            else:
                                wrt = wr0 if ox == -1 else wr2
                                wB = wrt[:, :, 1 + oz:1 + oz + ZC,
                                         1 + oy:1 + oy + YS]
                            prodT = prodTs[d % NR]
                            pview = prodT[:].rearrange(
                                "p (z y) i -> p i z y", z=ZC, y=YS)
                            dred = dreds[d % NR]
                            t1 = t1s[d % NR]
                            t2 = t2s[d % NR]
                            eng = dve if d % DOT_DVE_EVERY == 1 else gp
                            eng.tensor_tensor(out=pview, in0=wA, in1=wB,
                                              op=op.mult)
                            dve.tensor_reduce(out=dred[:], in_=prodT[:],
                                              axis=mybir.AxisListType.X,
                                              op=op.add)
                            Bv = B3[:, ox + 1,
                                    1 + zc + oz:1 + zc + oz + ZC,
                                    1 + oy:1 + oy + YS]
                            deng = gp if d % 2 else dve
                            deng.tensor_tensor(out=t1[:], in0=Bv, in1=cpv,
                                               op=op.add)
                            dve.scalar_tensor_tensor(out=t2[:], in0=dred[:],
                                                     scalar=-2.0, in1=t1[:],
                                                     op0=op.mult, op1=op.add)
                            deng.tensor_tensor(out=t1[:], in0=t2[:],
                                               in1=scv, op=op.mult)
                            act.activation(out=est[:, d], in_=t1[:],
                                           func=AF.Exp)

                        e9 = [dp.tile([128, ZC, YS], f16, name=f"e9_{r}", tag=f"e9_{r}")
                              for r in range(1, KN)]
                        e9b = [dp.tile([128, ZC, YS], f16, name=f"e9b_{r}", tag=f"e9b_{r}")
                               for r in range(1, KN)]
                        occ2s = [dp.tile([128, ZC, YS], f16,
                                         name=f"occ2_{i}", tag=f"occ2_{i}")
                                 for i in range(6)]
                        for r in range(1, KN):
                            idv = idx9[:, r, zc:zc + ZC, 1:1 + YS]
                            ach = gp if r in POOL_RANKS else dve
                            nd = 0
                            for d in range(27):
                                if d == 13:
                                    continue
                                first = nd < 2
                                er = e9[r - 1] if nd % 2 == 0 else e9b[r - 1]
                                nd += 1
                                oc = occ2s[(r + d) % 6]
                                if first:
                                    dve.scalar_tensor_tensor(
                                        out=er[:], in0=idv, scalar=float(d),
                                        in1=est[:, d], op0=op.is_equal,
                                        op1=op.mult)
                                else:
                                    dve.scalar_tensor_tensor(
                                        out=oc[:], in0=idv, scalar=float(d),
                                        in1=est[:, d], op0=op.is_equal,
                                        op1=op.mult)
                                    ach.tensor_tensor(out=er[:],
                                                      in0=er[:],
                                                      in1=oc[:], op=op.add)

                        for r in range(1, KN):
                            ach = gp if r in POOL_RANKS else dve
                            ach.tensor_tensor(out=e9[r - 1][:],
                                              in0=e9[r - 1][:],
                                              in1=e9b[r - 1][:], op=op.add)
                        ssum = dp.tile([128, ZC, YS], f16, tag="ssum")
                        sa = dp.tile([128, ZC, YS], f16, tag="sa")
                        sb = dp.tile([128, ZC, YS], f16, tag="sb")
                        sc = dp.tile([128, ZC, YS], f16, tag="sc")
                        sd = dp.tile([128, ZC, YS], f16, tag="sd")
                        dve.tensor_tensor(out=sa[:], in0=est[:, 13],
                                          in1=e9[0][:], op=op.add)
                        dve.tensor_tensor(out=sb[:], in0=e9[1][:],
                                          in1=e9[2][:], op=op.add)
                        dve.tensor_tensor(out=sc[:], in0=e9[3][:],
                                          in1=e9[4][:], op=op.add)
                        dve.tensor_tensor(out=sd[:], in0=e9[5][:],
                                          in1=e9[6][:], op=op.add)
                        dve.tensor_tensor(out=sa[:], in0=sa[:], in1=sb[:],
                                          op=op.add)
                        dve.tensor_tensor(out=sc[:], in0=sc[:], in1=sd[:],
                                          op=op.add)
                        dve.tensor_tensor(out=sa[:], in0=sa[:], in1=e9[7][:],
                                          op=op.add)
                        dve.tensor_tensor(out=ssum[:], in0=sa[:], in1=sc[:],
                                          op=op.add)
                        sf = dp.tile([128, ZC, YS], f32, tag="sf")
                        act.activation(out=sf[:], in_=ssum[:], func=AF.Copy)
                        recs = dp.tile([128, ZC, YS], f32, tag="recs")
                        dve.reciprocal(out=recs[:], in_=sf[:])
                        rec16 = dp.tile([128, ZC, YS], f16, tag="rec16")
                        act.activation(out=rec16[:], in_=recs[:], func=AF.Copy)

                        ob = dp.tile([128, ZC, YS, KN], f32, tag="ob")
                        dve.tensor_tensor(out=ob[:, :, :, 0], in0=est[:, 13],
                                          in1=rec16[:], op=op.mult)
                        for r in range(1, KN):
                            dve.tensor_tensor(out=ob[:, :, :, r],
                                              in0=e9[r - 1][:], in1=rec16[:],
                                              op=op.mult)
                        nc.sync.dma_start(out=outd[:, zc:zc + ZC], in_=ob[:])

    nc.compile()
    return nc


# --------------------------------------------------------------------------
# Host side
# --------------------------------------------------------------------------

_CACHED = {}


def _get_nc(ks_value):
    key = float(ks_value)
    if key not in _CACHED:
        _CACHED[key] = build_bass(key)
    return _CACHED[key]


def _shard_inputs(x):
    """x: [H, M, N] f32 -> list of per-core xin arrays [128, 3, ZE, YI]."""
    maps = []
    zext = np.arange(-1, H + 1) % H
    xs = np.arange(N)
    for c in range(NCORES):
        ys = (np.arange(YS * c - 2, YS * c + YS + 2)) % M
        slab = x[zext][:, ys, :]                       # [66, 20, 128]
        a = np.empty((128, 3, ZE, YI), dtype=np.float32)
        for r in range(3):
            xrot = (xs + r - 1) % N
            a[:, r] = slab[:, :, xrot].transpose(2, 0, 1)
        maps.append({"xin": np.ascontiguousarray(a)})
    return maps


def kernel(input, ksigma, k, w):
    from concourse.bass_utils import run_bass_kernel_spmd

    x = np.asarray(input, dtype=np.float32)
    assert x.shape == (H, M, N)
    ks = float(np.asarray(ksigma).reshape(-1)[0])
    assert int(k) == KN and int(w) == 3

    nc = _get_nc(ks)
    in_maps = _shard_inputs(x)
    res = run_bass_kernel_spmd(nc, in_maps, core_ids=list(range(NCORES)))
    full = np.empty((H, M, N, KN), dtype=np.float32)
    for c in range(NCORES):
        oc = res.results[c]["out"]          # [128, H, YS, KN]
        full[:, YS * c:YS * c + YS] = oc.transpose(1, 2, 0, 3)
    return full.reshape(H * M * N, KN)



# revision 3
# speedup vs baseline: 1.4076x; 1.4076x over previous
"""Trainium2 Bass kernel for nn_BuildK (27-neighborhood kNN softmax weights).

Strategy: shard the y dimension across 8 NeuronCores (spatial parallel, no
cross-core communication). Each core receives a halo-extended, x-rotated input
slab, computes per-voxel: the 9 intensity-nearest of its 27 periodic neighbors
(stable selection network on f32 |diff| keys with an index payload packed in
the low mantissa bits), reconstructs the sorted neighbor values via sign*dist,
forms the pairwise feature-row distances through 27 shifted dot planes, selects
the 9 logits belonging to the chosen neighbors, and applies a rowwise softmax.
Output is gathered and reassembled on the host.

All arithmetic is f32 on the DVE (ScalarE only for Exp); copies that only move
data go through DMA.
"""

import sys

sys.path.insert(0, "/opt/trn_rl_repo")

import numpy as np

H, M, N = 64, 128, 128
NCORES = 8
YS = M // NCORES          # 16 owned y rows per core
YE = YS + 2               # 18 = sort region (owned + 1 halo each side)
YI = YS + 4               # 20 = input slab y extent (halo 2)
ZE = H + 2                # 66 = z extent with periodic wrap rows
KN = 9
EPS = 1e-6
NSLOT = 36                # wire slots for the selection network


# --------------------------------------------------------------------------
# Selection network: top-9-sorted of 27, built from three 9-sorters and two
# pruned odd-even merges.  Ops are liveness-annotated for outputs 1..8
# (output 0 is always the center voxel: d=0, w=c).
# --------------------------------------------------------------------------

_SORT9 = [(0, 3), (1, 7), (2, 5), (4, 8), (0, 7), (2, 4), (3, 8), (5, 6),
          (0, 2), (1, 3), (4, 5), (7, 8), (1, 4), (3, 6), (5, 7), (0, 1),
          (2, 4), (3, 5), (6, 8), (2, 3), (4, 5), (6, 7), (1, 2), (3, 4),
          (5, 6)]


def _oddeven_merge(lo, n, r, out):
    step = r * 2
    if step < n:
        _oddeven_merge(lo, n, step, out)
        _oddeven_merge(lo + r, n, step, out)
        for i in range(lo + r, lo + n - r, step):
            out.append((i, i + r))
    else:
        out.append((lo, lo + r))


def _merge_topk(lenA, lenB, k):
    ces = []
    _oddeven_merge(0, 32, 1, ces)
    inf = [False] * 32
    for w in range(lenA, 16):
        inf[w] = True
    for w in range(16 + lenB, 32):
        inf[w] = True
    label = list(range(32))
    kept = []
    for (i, j) in ces:
        if inf[i] and inf[j]:
            continue
        if inf[j] and not inf[i]:
            continue
        if inf[i] and not inf[j]:
            label[i], label[j] = label[j], label[i]
            inf[i], inf[j] = False, True
            continue
        kept.append((label[i], label[j]))
    needed = set(label[w] for w in range(k))
    keep = []
    for (i, j) in reversed(kept):
        if i in needed or j in needed:
            keep.append((i, j))
            needed.add(i)
            needed.add(j)
    keep.reverse()

    def rm(w):
        return w if w < 16 else w - 16 + lenA

    return [(rm(i), rm(j)) for (i, j) in keep], [rm(label[w]) for w in range(k)]


def build_network():
    """Top-8-sorted of the 26 non-center candidates.  Wires 0..25 map to
    candidates CAND[w] (reference enumeration minus the center 13).
    Returns (ops, outw, cand): ops = [(i, j, need_i, need_j)]."""
    cand = [d for d in range(27) if d != 13]
    S8 = [(0, 1), (2, 3), (4, 5), (6, 7), (0, 2), (1, 3), (4, 6), (5, 7),
          (1, 2), (5, 6), (0, 4), (3, 7), (1, 5), (2, 6), (1, 4), (3, 6),
          (2, 4), (3, 5), (3, 4)]
    net = []
    net += [(i, j) for (i, j) in _SORT9]
    net += [(i + 9, j + 9) for (i, j) in _SORT9]
    net += [(i + 18, j + 18) for (i, j) in S8]
    m1, ow1 = _merge_topk(9, 9, 8)
    net += m1
    m2, ow2 = _merge_topk(8, 8, 8)
    remap = {i: ow1[i] for i in range(8)}
    remap.update({8 + i: 18 + i for i in range(8)})
    net += [(remap[i], remap[j]) for (i, j) in m2]
    outw = [remap[w] for w in ow2]

    live = set(outw)
    ops = []
    for (i, j) in reversed(net):
        ni, nj = i in live, j in live
        if not (ni or nj):
            continue
        ops.append((i, j, ni, nj))
        live.add(i)
        live.add(j)
    ops.reverse()
    return ops, outw, cand


NET_OPS, NET_OUTW, CAND = build_network()

OFFS = [(oz, oy, ox) for oz in (-1, 0, 1) for oy in (-1, 0, 1)
        for ox in (-1, 0, 1)]            # reference enumeration; 13 = center


# --------------------------------------------------------------------------
# Bass graph
# --------------------------------------------------------------------------

def build_bass(ks_value: float, reps: int = 1):
    from concourse import bacc, mybir
    from concourse import tile
    from concourse.alu_op_type import AluOpType as op

    f32 = mybir.dt.float32
    AF = mybir.ActivationFunctionType

    nc = bacc.Bacc("TRN2", target_bir_lowering=False, debug=False,
                   num_devices=NCORES)

    xin = nc.dram_tensor("xin", [128, 3, ZE, YI], f32, kind="ExternalInput").ap()
    outd = nc.dram_tensor("out", [128, H, YS, KN], f32,
                          kind="ExternalOutput").ap()

    ZC = 16                      # z chunk for both phases
    FS = ZC * YE                 # 288 free elems in sort phase

    dve = nc.vector
    act = nc.scalar

    with tile.TileContext(nc) as tc:
      for _rep in range(reps):
        with tc.tile_pool(name="pp", bufs=1) as pp:
            Wslab = pp.tile([128, KN, ZE, YE], f32, tag="Wslab")
            idx9 = pp.tile([128, KN - 1, H, YS], f32, tag="idx9")

            # ---------------- sort phase ----------------
            with tc.tile_pool(name="xp", bufs=1) as xp, \
                 tc.tile_pool(name="sortp", bufs=1) as sp:
                X3 = xp.tile([128, 3, ZE, YI], f32, tag="X3")
                nc.sync.dma_start(out=X3[:], in_=xin[:])
                kbig = sp.tile([128, NSLOT, FS], f32, tag="kbig")

                for zc in range(0, H, ZC):
                    cvw = X3[:, 1, 1 + zc:1 + zc + ZC, 1:1 + YE]

                    def vview(d):
                        oz, oy, ox = OFFS[d]
                        return X3[:, ox + 1,
                                  1 + zc + oz:1 + zc + oz + ZC,
                                  1 + oy:1 + oy + YE]

                    free_slots = list(range(NSLOT))
                    wire_slot = {}

                    def k_ap(s):
                        return kbig[:, s, :]

                    with tc.tile_pool(name="chunkp", bufs=1) as cp:
                        stmp = [cp.tile([128, FS], f32, name=f"s{i}", tag=f"s{i}")
                                for i in range(4)]
                        ptmp = [cp.tile([128, FS], f32, name=f"pt{i}",
                                        tag=f"pt{i}") for i in range(12)]
                        for w, d in enumerate(CAND):
                            s = free_slots.pop()
                            wire_slot[w] = s
                            st = stmp[d % 4]
                            dve.tensor_tensor(out=st[:], in0=vview(d),
                                              in1=cvw, op=op.subtract)
                            dd = ptmp[(3 * d) % 12]
                            c1 = ptmp[(3 * d + 1) % 12]
                            bb = ptmp[(3 * d + 2) % 12]
                            dve.scalar_tensor_tensor(
                                out=dd[:], in0=st[:], scalar=-1.0,
                                in1=st[:], op0=op.mult, op1=op.max)
                            dve.tensor_scalar(out=c1[:], in0=dd[:],
                                              scalar1=257.0, scalar2=None,
                                              op0=op.mult)
                            dve.tensor_tensor(out=dd[:], in0=c1[:],
                                              in1=dd[:], op=op.subtract)
                            dve.tensor_tensor(out=c1[:], in0=c1[:],
                                              in1=dd[:], op=op.subtract)
                            # c1 = hi; multiplier m = 1 + (2d+b)*2^-23
                            dve.tensor_scalar(out=bb[:], in0=st[:],
                                              scalar1=0.0, scalar2=None,
                                              op0=op.is_gt)
                            dve.tensor_scalar(
                                out=bb[:], in0=bb[:],
                                scalar1=float(np.float32(2.0**-23)),
                                scalar2=float(np.float32(1.0 + 2 * d * 2.0**-23)),
                                op0=op.mult, op1=op.add)
                            dve.tensor_tensor(out=k_ap(s), in0=c1[:],
                                              in1=bb[:], op=op.mult)

                        for n, (i, j, ni, nj) in enumerate(NET_OPS):
                            si, sj = wire_slot[i], wire_slot[j]
                            new_i = free_slots.pop() if ni else None
                            new_j = free_slots.pop() if nj else None
                            if ni:
                                dve.tensor_tensor(out=k_ap(new_i),
                                                  in0=k_ap(si), in1=k_ap(sj),
                                                  op=op.min)
                            if nj:
                                dve.tensor_tensor(out=k_ap(new_j),
                                                  in0=k_ap(si), in1=k_ap(sj),
                                                  op=op.max)
                            free_slots.append(si)
                            free_slots.append(sj)
                            if ni:
                                wire_slot[i] = new_i
                            else:
                                del wire_slot[i]
                            if nj:
                                wire_slot[j] = new_j
                            else:
                                del wire_slot[j]

                        # unpack sorted keys: hi (distance), delta, sign;
                        # reconstruct w_r = c + sign*hi directly
                        dsor = [cp.tile([128, FS], f32, name=f"ds{r}",
                                        tag=f"ds{r}") for r in range(8)]
                        ROUND_C = float(1.5 * 2.0**23)
                        for r in range(1, KN):
                            pk = k_ap(wire_slot[NET_OUTW[r - 1]])
                            e1 = ptmp[(3 * r) % 12]
                            e2 = ptmp[(3 * r + 1) % 12]
                            e3 = ptmp[(3 * r + 2) % 12]
                            hi = dsor[r - 1]
                            dve.tensor_scalar(out=e1[:], in0=pk,
                                              scalar1=257.0, scalar2=None,
                                              op0=op.mult)
                            dve.tensor_tensor(out=e2[:], in0=e1[:], in1=pk,
                                              op=op.subtract)
                            dve.tensor_tensor(out=hi[:], in0=e1[:],
                                              in1=e2[:], op=op.subtract)
                            dve.tensor_tensor(out=e1[:], in0=pk, in1=hi[:],
                                              op=op.subtract)
                            dve.tensor_scalar(out=e3[:], in0=hi[:],
                                              scalar1=1e-30, scalar2=None,
                                              op0=op.add)
                            dve.reciprocal(out=e2[:], in_=e3[:])
                            dve.tensor_tensor(out=e1[:], in0=e1[:],
                                              in1=e2[:], op=op.mult)
                            dve.tensor_scalar(out=e1[:], in0=e1[:],
                                              scalar1=float(2.0**23),
                                              scalar2=ROUND_C, op0=op.mult,
                                              op1=op.add)
                            # e1 = code + ROUND_C ; code = 2*delta + signbit
                            dve.tensor_scalar(out=e2[:], in0=e1[:],
                                              scalar1=ROUND_C, scalar2=None,
                                              op0=op.subtract)
                            # delta = round((code - 0.5) * 0.5)
                            dve.tensor_scalar(out=e3[:], in0=e2[:],
                                              scalar1=-0.5, scalar2=0.5,
                                              op0=op.add, op1=op.mult)
                            dve.tensor_scalar(out=e3[:], in0=e3[:],
                                              scalar1=ROUND_C, scalar2=None,
                                              op0=op.add)
                            dve.tensor_scalar(out=e3[:], in0=e3[:],
                                              scalar1=ROUND_C, scalar2=None,
                                              op0=op.subtract)
                            # e3 = delta (dir index); store owned y cols only
                            e3v = e3[:].rearrange("p (z y) -> p z y",
                                                  z=ZC, y=YE)
                            dve.tensor_copy(
                                out=idx9[:, r - 1, zc:zc + ZC, :],
                                in_=e3v[:, :, 1:1 + YS])
                            # sign = (code - 2*delta)*2 - 1
                            dve.scalar_tensor_tensor(out=e1[:], in0=e3[:],
                                                     scalar=-2.0, in1=e2[:],
                                                     op0=op.mult, op1=op.add)
                            dve.tensor_scalar(out=e1[:], in0=e1[:],
                                              scalar1=2.0, scalar2=-1.0,
                                              op0=op.mult, op1=op.add)
                            # w_r = c + sign*hi
                            dve.tensor_tensor(out=e2[:], in0=e1[:],
                                              in1=hi[:], op=op.mult)
                            dve.tensor_tensor(
                                out=Wslab[:, r, 1 + zc:1 + zc + ZC, :],
                                in0=e2[:], in1=cvw, op=op.add)

                        dve.tensor_copy(
                            out=Wslab[:, 0, 1 + zc:1 + zc + ZC, :],
                            in_=cvw)

            # ---------------- z wrap rows of Wslab ----------------
            nc.sync.dma_start(out=Wslab[:, :, 0:1, :],
                              in_=Wslab[:, :, H:H + 1, :])
            nc.sync.dma_start(out=Wslab[:, :, ZE - 1:ZE, :],
                              in_=Wslab[:, :, 1:2, :])

            # ---------------- sigma / scale planes ----------------
            with tc.tile_pool(name="spp", bufs=1) as spp:
                B3 = spp.tile([128, 3, ZE, YE], f32, tag="B3")
                scalem = spp.tile([128, H, YS], f32, tag="scalem")
                Cp = spp.tile([128, H, YS], f32, tag="Cp")

                with tc.tile_pool(name="sigt", bufs=1) as sg2:
                    S1 = sg2.tile([128, ZE, YE], f32, tag="S1")
                    S2 = sg2.tile([128, ZE, YE], f32, tag="S2")
                    sq = sg2.tile([128, ZE, YE], f32, tag="sq")
                    dve.tensor_tensor(out=S1[:], in0=Wslab[:, 0],
                                      in1=Wslab[:, 1], op=op.add)
                    for r in range(2, KN):
                        dve.tensor_tensor(out=S1[:], in0=S1[:],
                                          in1=Wslab[:, r], op=op.add)
                    dve.tensor_tensor(out=S2[:], in0=Wslab[:, 0],
                                      in1=Wslab[:, 0], op=op.mult)
                    for r in range(1, KN):
                        dve.tensor_tensor(out=sq[:], in0=Wslab[:, r],
                                          in1=Wslab[:, r], op=op.mult)
                        dve.tensor_tensor(out=S2[:], in0=S2[:], in1=sq[:],
                                          op=op.add)
                    dve.scalar_tensor_tensor(out=B3[:, 1], in0=S1[:],
                                             scalar=-2.0 * EPS, in1=S2[:],
                                             op0=op.mult, op1=op.add)
                    nc.sync.dma_start(out=B3[:, 0][1:128], in_=B3[:, 1][0:127])
                    nc.sync.dma_start(out=B3[:, 0][0:1], in_=B3[:, 1][127:128])
                    nc.sync.dma_start(out=B3[:, 2][0:127], in_=B3[:, 1][1:128])
                    nc.sync.dma_start(out=B3[:, 2][127:128], in_=B3[:, 1][0:1])

                    S1o = S1[:, 1:1 + H, 1:1 + YS]
                    S2o = S2[:, 1:1 + H, 1:1 + YS]
                    sq1 = sg2.tile([128, H, YS], f32, tag="sq1")
                    tvar = sg2.tile([128, H, YS], f32, tag="tvar")
                    tmpv = sg2.tile([128, H, YS], f32, tag="tmpv")
                    rec = sg2.tile([128, H, YS], f32, tag="rec")
                    dve.tensor_tensor(out=sq1[:], in0=S1o, in1=S1o,
                                      op=op.mult)
                    dve.scalar_tensor_tensor(out=tvar[:], in0=sq1[:],
                                             scalar=-1.0 / 9.0, in1=S2o,
                                             op0=op.mult, op1=op.add)
                    dve.tensor_scalar(out=tmpv[:], in0=tvar[:], scalar1=0.0,
                                      scalar2=None, op0=op.is_equal)
                    dve.tensor_tensor(out=tmpv[:], in0=tmpv[:], in1=tvar[:],
                                      op=op.add)
                    dve.reciprocal(out=rec[:], in_=tmpv[:])
                    dve.tensor_scalar(out=rec[:], in0=rec[:],
                                      scalar1=-4.0 / (ks_value * ks_value),
                                      scalar2=None, op0=op.mult)
                    dve.tensor_scalar(out=tmpv[:], in0=tvar[:], scalar1=0.0,
                                      scalar2=None, op0=op.not_equal)
                    dve.tensor_tensor(out=scalem[:], in0=rec[:], in1=tmpv[:],
                                      op=op.mult)
                    dve.scalar_tensor_tensor(out=Cp[:], in0=S1o,
                                             scalar=2.0 * EPS, in1=S2o,
                                             op0=op.mult, op1=op.add)
                    dve.tensor_scalar(out=Cp[:], in0=Cp[:],
                                      scalar1=9.0 * EPS * EPS, scalar2=None,
                                      op0=op.add)

                # ---------------- dots + logit-select + softmax ----------
                with tc.tile_pool(name="dotp", bufs=1) as dp:
                    for zc in range(0, H, ZC):
                        wr0 = dp.tile([128, KN, ZC + 2, YE], f32, tag="wr0")
                        wr2 = dp.tile([128, KN, ZC + 2, YE], f32, tag="wr2")
                        src = Wslab[:, :, zc:zc + ZC + 2, :]
                        nc.sync.dma_start(out=wr0[1:128], in_=src[0:127])
                        nc.sync.dma_start(out=wr0[0:1], in_=src[127:128])
                        nc.sync.dma_start(out=wr2[0:127], in_=src[1:128])
                        nc.sync.dma_start(out=wr2[127:128], in_=src[0:1])

                        est = dp.tile([128, 27, ZC, YS], f32, tag="est")
                        prodT = dp.tile([128, ZC * YS, KN], f32, tag="prodT")
                        dred = dp.tile([128, ZC, YS], f32, tag="dred")
                        t1 = dp.tile([128, ZC, YS], f32, tag="t1")
                        scv = scalem[:, zc:zc + ZC, :]
                        cpv = Cp[:, zc:zc + ZC, :]

                        wA = Wslab[:, :, 1 + zc:1 + zc + ZC, 1:1 + YS]
                        for d in range(27):
                            oz, oy, ox = OFFS[d]
                            if ox == 0:
                                wB = Wslab[:, :,
                                           1 + zc + oz:1 + zc + oz + ZC,
                                           1 + oy:1 + oy + YS]
                            else:
                                wrt = wr0 if ox == -1 else wr2
                                wB = wrt[:, :, 1 + oz:1 + oz + ZC,
                                         1 + oy:1 + oy + YS]
                            pview = prodT[:].rearrange(
                                "p (z y) i -> p i z y", z=ZC, y=YS)
                            dve.tensor_tensor(out=pview, in0=wA, in1=wB,
                                              op=op.mult)
                            dve.tensor_reduce(out=dred[:], in_=prodT[:],
                                              axis=mybir.AxisListType.X,
                                              op=op.add)
                            Bv = B3[:, ox + 1,
                                    1 + zc + oz:1 + zc + oz + ZC,
                                    1 + oy:1 + oy + YS]
                            dve.tensor_tensor(out=t1[:], in0=Bv, in1=cpv,
                                              op=op.add)
                            dve.scalar_tensor_tensor(out=t1[:], in0=dred[:],
                                                     scalar=-2.0, in1=t1[:],
                                                     op0=op.mult, op1=op.add)
                            dve.tensor_tensor(out=est[:, d], in0=t1[:],
                                              in1=scv, op=op.mult)

                        # select the logit of the chosen neighbor per rank
                        esel = [dp.tile([128, ZC, YS], f32, name=f"esel{r}",
                                        tag=f"esel{r}") for r in range(1, KN)]
                        oc = dp.tile([128, ZC, YS], f32, tag="oc")
                        for r in range(1, KN):
                            idv = idx9[:, r - 1, zc:zc + ZC, :]
                            er = esel[r - 1]
                            first = True
                            for d in range(27):
                                if d == 13:
                                    continue
                                if first:
                                    dve.scalar_tensor_tensor(
                                        out=er[:], in0=idv, scalar=float(d),
                                        in1=est[:, d], op0=op.is_equal,
                                        op1=op.mult)
                                    first = False
                                else:
                                    dve.scalar_tensor_tensor(
                                        out=oc[:], in0=idv, scalar=float(d),
                                        in1=est[:, d], op0=op.is_equal,
                                        op1=op.mult)
                                    dve.tensor_tensor(out=er[:], in0=er[:],
                                                      in1=oc[:], op=op.add)

                        # exp of the 9 selected logits, rowwise softmax
                        eexp = [dp.tile([128, ZC, YS], f32, name=f"ee{r}",
                                        tag=f"ee{r}") for r in range(KN)]
                        act.activation(out=eexp[0][:], in_=est[:, 13],
                                       func=AF.Exp)
                        for r in range(1, KN):
                            act.activation(out=eexp[r][:],
                                           in_=esel[r - 1][:], func=AF.Exp)

                        sa = dp.tile([128, ZC, YS], f32, tag="sa")
                        sb = dp.tile([128, ZC, YS], f32, tag="sb")
                        sc = dp.tile([128, ZC, YS], f32, tag="sc")
                        sd = dp.tile([128, ZC, YS], f32, tag="sd")
                        dve.tensor_tensor(out=sa[:], in0=eexp[0][:],
                                          in1=eexp[1][:], op=op.add)
                        dve.tensor_tensor(out=sb[:], in0=eexp[2][:],
                                          in1=eexp[3][:], op=op.add)
                        dve.tensor_tensor(out=sc[:], in0=eexp[4][:],
                                          in1=eexp[5][:], op=op.add)
                        dve.tensor_tensor(out=sd[:], in0=eexp[6][:],
                                          in1=eexp[7][:], op=op.add)
                        dve.tensor_tensor(out=sa[:], in0=sa[:], in1=sb[:],
                                          op=op.add)
                        dve.tensor_tensor(out=sc[:], in0=sc[:], in1=sd[:],
                                          op=op.add)
                        dve.tensor_tensor(out=sa[:], in0=sa[:],
                                          in1=eexp[8][:], op=op.add)
                        dve.tensor_tensor(out=sa[:], in0=sa[:], in1=sc[:],
                                          op=op.add)
                        recs = dp.tile([128, ZC, YS], f32, tag="recs")
                        dve.reciprocal(out=recs[:], in_=sa[:])

                        ob = dp.tile([128, ZC, YS, KN], f32, tag="ob")
                        for r in range(KN):
                            dve.tensor_tensor(out=ob[:, :, :, r],
                                              in0=eexp[r][:], in1=recs[:],
                                              op=op.mult)
                        nc.sync.dma_start(out=outd[:, zc:zc + ZC], in_=ob[:])

    nc.compile()
    return nc


# --------------------------------------------------------------------------
# Host side
# --------------------------------------------------------------------------

_CACHED = {}


def _get_nc(ks_value):
    key = float(ks_value)
    if key not in _CACHED:
        _CACHED[key] = build_bass(key)
    return _CACHED[key]


def _shard_inputs(x):
    """x: [H, M, N] f32 -> list of per-core xin arrays [128, 3, ZE, YI]."""
    maps = []
    zext = np.arange(-1, H + 1) % H
    xs = np.arange(N)
    for c in range(NCORES):
        ys = (np.arange(YS * c - 2, YS * c + YS + 2)) % M
        slab = x[zext][:, ys, :]                       # [66, 20, 128]
        a = np.empty((128, 3, ZE, YI), dtype=np.float32)
        for r in range(3):
            xrot = (xs + r - 1) % N
            a[:, r] = slab[:, :, xrot].transpose(2, 0, 1)
        maps.append({"xin": np.ascontiguousarray(a)})
    return maps


def kernel(input, ksigma, k, w):
    from concourse.bass_utils import run_bass_kernel_spmd

    x = np.asarray(input, dtype=np.float32)
    assert x.shape == (H, M, N)
    ks = float(np.asarray(ksigma).reshape(-1)[0])
    assert int(k) == KN and int(w) == 3

    nc = _get_nc(ks)
    in_maps = _shard_inputs(x)
    res = run_bass_kernel_spmd(nc, in_maps, core_ids=list(range(NCORES)))
    full = np.empty((H, M, N, KN), dtype=np.float32)
    for c in range(NCORES):
        oc = res.results[c]["out"]          # [128, H, YS, KN]
        full[:, YS * c:YS * c + YS] = oc.transpose(1, 2, 0, 3)
    return full.reshape(H * M * N, KN)


# revision 11
# speedup vs baseline: 5.0736x; 3.6045x over previous
"""Trainium2 Bass kernel for nn_BuildK (27-neighborhood kNN softmax weights).

v2 design, tuned for the axon backend cost model (per-instruction overhead
dominates): minimize instruction count.

- Sort phase (2 z-chunks of 32): 27 neighbor keys packed as
  trunc(|diff|) * (1 + (2d+sign)*2^-23) in a contiguous [32, FS] tile,
  sorted by a batched Batcher odd-even mergesort (each network level's
  comparator groups run as single strided-AP min/max instructions).
  Decode of the 8 nearest runs batched over ranks. Results (neighbor
  values Wd, neighbor dir indices Id) are staged to DRAM scratch.
- Sigma phase: rowwise unbiased variance -> logit scale.
- Dot phase (2 z-chunks of 32): pairwise feature distances via the exact
  symmetry dist2(v,d) = dist2(v+off_d, 26-d); 13 extended difference
  planes (eps dropped - well within tolerance), per-dir logits selected
  into rank slots with broadcast index-compare ops, single Exp, softmax.
"""

import sys

sys.path.insert(0, "/opt/trn_rl_repo")

import numpy as np

H, M, N = 64, 128, 128
NCORES = 8
YS = M // NCORES          # 16 owned y rows per core
YE = YS + 2               # 18 = sort region (owned + 1 halo each side)
YI = YS + 4               # 20 = input slab y extent (halo 2)
ZE = H + 2                # 66 = z extent with periodic wrap rows
KN = 9
SZ = 32                   # z chunk
FS = SZ * YE              # 576 free elems per sort row

OFFS = [(oz, oy, ox) for oz in (-1, 0, 1) for oy in (-1, 0, 1)
        for ox in (-1, 0, 1)]            # reference enumeration; 13 = center


# --------------------------------------------------------------------------
# Batched Batcher odd-even mergesort schedule for 32 rows.
# Groups: (base, d1, n1, d2, n2, r) -> compare rows (i, i+r),
# i = base + a*d1 + b*d2.  Groups capped at 8 pairs (scratch size).
# --------------------------------------------------------------------------

def _oddeven_comparators(n):
    ops = []

    def merge(lo, m, r):
        step = r * 2
        if step < m:
            merge(lo, m, step)
            merge(lo + r, m, step)
            for i in range(lo + r, lo + m - r, step):
                ops.append((i, i + r))
        else:
            ops.append((lo, lo + r))

    def srt(lo, m):
        if m > 1:
            h = m // 2
            srt(lo, h)
            srt(lo + h, h)
            merge(lo, m, 1)

    srt(0, n)
    return ops


def _grid_decompose(idxs):
    idxs = sorted(idxs)
    grids = []
    rest = idxs
    while rest:
        if len(rest) == 1:
            grids.append((rest[0], 1, 1, 1, 1))
            break
        d1 = rest[1] - rest[0]
        runs = []
        s = rest[0]
        cnt = 1
        for a, b in zip(rest, rest[1:]):
            if b - a == d1:
                cnt += 1
            else:
                runs.append((s, cnt))
                s = b
                cnt = 1
        runs.append((s, cnt))
        n1 = min(c for (_, c) in runs)
        starts = []
        leftover = []
        for (st, c) in runs:
            starts.append(st)
            if c > n1:
                leftover.extend(range(st + n1 * d1, st + c * d1, d1))
        ok2 = True
        d2 = starts[1] - starts[0] if len(starts) > 1 else 1
        for a, b in zip(starts, starts[1:]):
            if b - a != d2:
                ok2 = False
        if ok2:
            grids.append((starts[0], d1, n1, d2, len(starts)))
            rest = sorted(leftover)
        else:
            st, c = runs[0]
            grids.append((st, d1, c, 1, 1))
            rest = sorted(set(rest) - set(range(st, st + c * d1, d1)))
    return grids


def batched_schedule(n=32, maxpairs=8):
    ops = _oddeven_comparators(n)
    level = [0] * n
    lv = []
    for (i, j) in ops:
        l = max(level[i], level[j])
        lv.append((l, i, j))
        level[i] = l + 1
        level[j] = l + 1
    from collections import defaultdict
    bylvr = defaultdict(list)
    for (l, i, j) in lv:
        bylvr[(l, j - i)].append(i)
    sched = []
    for (l, r) in sorted(bylvr.keys()):
        for (base, d1, n1, d2, n2) in _grid_decompose(bylvr[(l, r)]):
            # split so n1*n2 <= maxpairs (split the bigger factor)
            cells = [(base + b * d2, d1, n1) for b in range(n2)]
            cur = []
            cnt = 0
            for (b0, dd1, nn1) in cells:
                while nn1 > 0:
                    take = min(nn1, maxpairs - cnt)
                    cur.append((b0, dd1, take))
                    b0 += take * dd1
                    nn1 -= take
                    cnt += take
                    if cnt == maxpairs:
                        sched.append((cur, r))
                        cur = []
                        cnt = 0
            if cur:
                sched.append((cur, r))
    # each entry: (list of (base, d1, n1) runs, r); regroup runs into
    # (base, d1, n1, d2, n2) when runs are evenly spaced with equal n1
    out = []
    for (runs, r) in sched:
        if len(runs) == 1:
            b, d1, n1 = runs[0]
            out.append((b, d1, n1, 1, 1, r))
        else:
            n1s = set(x[2] for x in runs)
            d1s = set(x[1] for x in runs)
            bs = [x[0] for x in runs]
            gaps = set(b2 - b1 for b1, b2 in zip(bs, bs[1:]))
            if len(n1s) == 1 and len(d1s) == 1 and len(gaps) == 1:
                out.append((bs[0], d1s.pop(), n1s.pop(), gaps.pop(),
                            len(bs), r))
            else:
                for (b, d1, n1) in runs:
                    out.append((b, d1, n1, 1, 1, r))
    return out


SORT_SCHED = batched_schedule(32, maxpairs=8)

PREP_PACKS = [list(range(0, 4)), list(range(4, 8)), [8]]  # groups of (oz,oy)


# --------------------------------------------------------------------------
# Bass graph
# --------------------------------------------------------------------------

def build_bass(ks_value: float, reps: int = 1):
    from concourse import bacc, mybir
    from concourse import tile
    from concourse.alu_op_type import AluOpType as op
    from concourse.bass_types import AP

    f32 = mybir.dt.float32
    AF = mybir.ActivationFunctionType

    nc = bacc.Bacc("TRN2", target_bir_lowering=False, debug=False,
                   num_devices=NCORES)

    xin = nc.dram_tensor("xin", [128, 3, ZE, YI], f32,
                         kind="ExternalInput").ap()
    cst = nc.dram_tensor("cst", [128, 27, 1], f32, kind="ExternalInput").ap()
    outd = nc.dram_tensor("out", [128, KN, H, YS], f32,
                          kind="ExternalOutput").ap()
    Wd = nc.dram_tensor("Wd", [128, KN, ZE, YE], f32, kind="Internal").ap()
    Id = nc.dram_tensor("Id", [128, 8, H, YS], f32, kind="Internal").ap()

    dve = nc.vector
    act = nc.scalar

    def bcast0(ap, n):
        """Insert a stride-0 dim of length n right after the partition dim."""
        pairs = [list(p) for p in ap.ap]
        newpairs = [pairs[0], [0, n]] + pairs[1:]
        return AP(ap.tensor, ap.offset, newpairs)

    def rows_ap(tile_handle_ap, R, F, base, d1, n1, d2, n2, foff=0, flen=None):
        """AP selecting rows {base + a*d1 + b*d2} of a [128, R, F] tile."""
        if flen is None:
            flen = F
        pairs = [[R * F, 128]]
        if n2 > 1:
            pairs.append([d2 * F, n2])
        if n1 > 1:
            pairs.append([d1 * F, n1])
        pairs.append([1, flen])
        return AP(tile_handle_ap.tensor, tile_handle_ap.offset + base * F + foff,
                  pairs)

    with tile.TileContext(nc) as tc:
      for _rep in range(reps):
        with tc.tile_pool(name="pp", bufs=1) as pp:
            scalem = pp.tile([128, H, YS], f32, tag="scalem")
            cstt = pp.tile([128, 27, 1], f32, tag="cstt")
            nc.sync.dma_start(out=cstt[:], in_=cst[:])

            # ================= sort phase =================
            for zc in (0, SZ):
                with tc.tile_pool(name="sortp", bufs=1) as sp:
                    X3c = sp.tile([128, 3, SZ + 2, YI], f32, tag="X3c")
                    nc.sync.dma_start(out=X3c[:],
                                      in_=xin[:, :, zc:zc + SZ + 2, :])
                    K = sp.tile([128, 32, FS], f32, tag="K")
                    T1 = sp.tile([128, 12, FS], f32, tag="T1")
                    T2 = sp.tile([128, 12, FS], f32, tag="T2")
                    T3 = sp.tile([128, 12, FS], f32, tag="T3")
                    SCR = sp.tile([128, 8, FS], f32, tag="SCR")
                    Wstage = sp.tile([128, KN, SZ, YE], f32, tag="Wstage")

                    cvw = X3c[:, 1, 1:1 + SZ, 1:1 + YE]     # [32,18]

                    dve.memset(K[:, 27:32], 3.0e38)

                    # ---- prep: keys for all 27 dirs ----
                    for pack in PREP_PACKS:
                        r0 = pack[0] * 3
                        nr = len(pack) * 3
                        for jj, g in enumerate(pack):
                            oz, oy = g // 3 - 1, g % 3 - 1
                            vv = X3c[:, :, 1 + oz:1 + oz + SZ,
                                     1 + oy:1 + oy + YE]
                            dve.tensor_tensor(
                                out=T1[:, jj * 3:jj * 3 + 3],
                                in0=vv, in1=bcast0(cvw, 3), op=op.subtract)
                        t1 = T1[:, 0:nr]
                        t2 = T2[:, 0:nr]
                        t3 = T3[:, 0:nr]
                        dve.scalar_tensor_tensor(out=t2, in0=t1, scalar=-1.0,
                                                 in1=t1, op0=op.mult,
                                                 op1=op.max)
                        dve.tensor_scalar(out=t3, in0=t2, scalar1=257.0,
                                          scalar2=None, op0=op.mult)
                        dve.tensor_tensor(out=t2, in0=t3, in1=t2,
                                          op=op.subtract)
                        dve.tensor_tensor(out=t3, in0=t3, in1=t2,
                                          op=op.subtract)       # t3 = hi
                        dve.tensor_scalar(out=t2, in0=t1, scalar1=0.0,
                                          scalar2=None, op0=op.is_gt)
                        cr = cstt[:, r0:r0 + nr, :]
                        dve.scalar_tensor_tensor(
                            out=t1, in0=t2, scalar=float(np.float32(2.0**-23)),
                            in1=cr.broadcast_to((128, nr, FS)),
                            op0=op.mult, op1=op.add)            # t1 = m
                        dve.tensor_tensor(out=K[:, r0:r0 + nr], in0=t3,
                                          in1=t1, op=op.mult)

                    # ---- batched Batcher sort ----
                    for (base, d1, n1, d2, n2, r) in SORT_SCHED:
                        npairs = n1 * n2
                        lo = rows_ap(K[:], 32, FS, base, d1, n1, d2, n2)
                        hi = rows_ap(K[:], 32, FS, base + r, d1, n1, d2, n2)
                        sc = SCR[:, 0:npairs]
                        dve.tensor_tensor(out=sc, in0=lo, in1=hi, op=op.min)
                        dve.tensor_tensor(out=hi, in0=lo, in1=hi, op=op.max)
                        dve.tensor_copy(out=lo, in_=sc)

                    # ---- batched decode of ranks 1..8 ----
                    KS = K[:, 1:9]
                    e1 = T1[:, 0:8]
                    e2 = T2[:, 0:8]
                    e3 = T3[:, 0:8]
                    sc = SCR[:]
                    C = float(1.5 * 2.0**23)
                    dve.tensor_scalar(out=e1, in0=KS, scalar1=257.0,
                                      scalar2=None, op0=op.mult)
                    dve.tensor_tensor(out=e2, in0=e1, in1=KS, op=op.subtract)
                    dve.tensor_tensor(out=e3, in0=e1, in1=e2, op=op.subtract)
                    dve.tensor_tensor(out=e1, in0=KS, in1=e3, op=op.subtract)
                    dve.tensor_scalar(out=e2, in0=e3, scalar1=1e-30,
                                      scalar2=None, op0=op.add)
                    dve.reciprocal(out=sc, in_=e2)
                    dve.tensor_tensor(out=e1, in0=e1, in1=sc, op=op.mult)
                    dve.tensor_scalar(out=e1, in0=e1, scalar1=float(2.0**23),
                                      scalar2=C, op0=op.mult, op1=op.add)
                    dve.tensor_scalar(out=e2, in0=e1, scalar1=C, scalar2=None,
                                      op0=op.subtract)          # code
                    dve.tensor_scalar(out=e1, in0=e2, scalar1=-0.5,
                                      scalar2=0.5, op0=op.add, op1=op.mult)
                    dve.tensor_scalar(out=e1, in0=e1, scalar1=C, scalar2=None,
                                      op0=op.add)
                    dve.tensor_scalar(out=e1, in0=e1, scalar1=C, scalar2=None,
                                      op0=op.subtract)          # e1 = delta
                    # idx -> contiguous staging in SCR, then DRAM
                    d_view = AP(T1[:].tensor, T1[:].offset,
                                [[12 * FS, 128], [FS, 8], [YE, SZ], [1, YS]])
                    d_view = AP(d_view.tensor, d_view.offset + 1, d_view.ap)
                    idq = AP(SCR[:].tensor, SCR[:].offset,
                             [[8 * FS, 128], [SZ * YS, 8], [1, SZ * YS]])
                    dve.tensor_copy(out=idq, in_=d_view)
                    nc.sync.dma_start(out=Id[:, :, zc:zc + SZ, :], in_=idq)
                    # sign & neighbor values
                    dve.scalar_tensor_tensor(out=sc, in0=e1, scalar=-2.0,
                                             in1=e2, op0=op.mult, op1=op.add)
                    dve.tensor_scalar(out=sc, in0=sc, scalar1=2.0,
                                      scalar2=-1.0, op0=op.mult, op1=op.add)
                    dve.tensor_tensor(out=sc, in0=sc, in1=e3, op=op.mult)
                    dve.tensor_tensor(out=Wstage[:, 1:9], in0=sc,
                                      in1=bcast0(cvw, 8), op=op.add)
                    dve.tensor_copy(out=Wstage[:, 0], in_=cvw)
                    nc.sync.dma_start(out=Wd[:, :, 1 + zc:1 + zc + SZ, :],
                                      in_=Wstage[:])
                    # z wrap rows of Wd, straight from the staging tile
                    if zc == 0:
                        nc.sync.dma_start(out=Wd[:, :, ZE - 1:ZE, :],
                                          in_=Wstage[:, :, 0:1, :])
                    else:
                        nc.sync.dma_start(out=Wd[:, :, 0:1, :],
                                          in_=Wstage[:, :, SZ - 1:SZ, :])

            # ================= sigma phase =================
            with tc.tile_pool(name="sigp", bufs=1) as sg:
                Wf = sg.tile([128, KN, ZE, YE], f32, tag="Wf")
                nc.sync.dma_start(out=Wf[:], in_=Wd[:])
                Wo = Wf[:, :, 1:1 + H, 1:1 + YS]      # [9,64,16]
                t4 = sg.tile([128, 4, H, YS], f32, tag="t4")
                sq9 = sg.tile([128, KN, H, YS], f32, tag="sq9")
                S1 = sg.tile([128, H, YS], f32, tag="S1")
                S2 = sg.tile([128, H, YS], f32, tag="S2")
                tv = sg.tile([128, H, YS], f32, tag="tv")
                dve.tensor_tensor(out=t4[:], in0=Wf[:, 0:4, 1:1 + H, 1:1 + YS],
                                  in1=Wf[:, 4:8, 1:1 + H, 1:1 + YS], op=op.add)
                dve.tensor_tensor(out=t4[:, 0:2], in0=t4[:, 0:2],
                                  in1=t4[:, 2:4], op=op.add)
                dve.tensor_tensor(out=t4[:, 0], in0=t4[:, 0], in1=t4[:, 1],
                                  op=op.add)
                dve.tensor_tensor(out=S1[:], in0=t4[:, 0],
                                  in1=Wf[:, 8, 1:1 + H, 1:1 + YS], op=op.add)
                dve.tensor_tensor(out=sq9[:], in0=Wo, in1=Wo, op=op.mult)
                dve.tensor_tensor(out=t4[:], in0=sq9[:, 0:4], in1=sq9[:, 4:8],
                                  op=op.add)
                dve.tensor_tensor(out=t4[:, 0:2], in0=t4[:, 0:2],
                                  in1=t4[:, 2:4], op=op.add)
                dve.tensor_tensor(out=t4[:, 0], in0=t4[:, 0], in1=t4[:, 1],
                                  op=op.add)
                dve.tensor_tensor(out=S2[:], in0=t4[:, 0], in1=sq9[:, 8],
                                  op=op.add)
                dve.tensor_tensor(out=tv[:], in0=S1[:], in1=S1[:], op=op.mult)
                dve.scalar_tensor_tensor(out=tv[:], in0=tv[:],
                                         scalar=-1.0 / 9.0, in1=S2[:],
                                         op0=op.mult, op1=op.add)  # tvar
                dve.tensor_scalar(out=S1[:], in0=tv[:], scalar1=0.0,
                                  scalar2=None, op0=op.is_equal)
                dve.tensor_tensor(out=S1[:], in0=S1[:], in1=tv[:], op=op.add)
                dve.reciprocal(out=S2[:], in_=S1[:])
                dve.tensor_scalar(out=S2[:], in0=S2[:],
                                  scalar1=-4.0 / (ks_value * ks_value),
                                  scalar2=None, op0=op.mult)
                dve.tensor_scalar(out=S1[:], in0=tv[:], scalar1=0.0,
                                  scalar2=None, op0=op.not_equal)
                dve.tensor_tensor(out=scalem[:], in0=S2[:], in1=S1[:],
                                  op=op.mult)

            # ================= dot phase =================
            DRD = [d for d in range(13)]
            for zc in (0, SZ):
                with tc.tile_pool(name="dotp", bufs=1) as dp:
                    Wm = dp.tile([128, KN, SZ + 2, YE], f32, tag="Wm")
                    nc.sync.dma_start(out=Wm[:],
                                      in_=Wd[:, :, zc:zc + SZ + 2, :])
                    wrb = dp.tile([128, KN, SZ + 2, YE], f32, tag="wrb")
                    idxc = dp.tile([128, 8, SZ * YS], f32, tag="idxc")
                    nc.sync.dma_start(out=idxc[:],
                                      in_=Id[:, :, zc:zc + SZ, :])
                    diff = dp.tile([128, KN, SZ + 1, 17], f32, tag="diff")
                    drx = [dp.tile([128, SZ + 1, 17], f32, name=f"drx{d}",
                                   tag=f"drx{d}") for d in range(13)]
                    rot = [dp.tile([128, SZ + 1, 17], f32, name=f"rot{i}",
                                   tag=f"rot{i}") for i in range(2)]
                    L = dp.tile([128, SZ * YS], f32, tag="L")
                    es8 = dp.tile([128, 8, SZ * YS], f32, tag="es8")
                    oc8 = dp.tile([128, 8, SZ * YS], f32, tag="oc8")
                    ee9 = dp.tile([128, KN, SZ * YS], f32, tag="ee9")
                    s4 = dp.tile([128, 4, SZ * YS], f32, tag="s4")

                    # 13 extended squared-distance planes, grouped by ox so
                    # one rotated slab buffer suffices
                    def do_dist(d, wbt):
                        oz, oy, ox = OFFS[d]
                        y0 = -1 if oy == 1 else 0
                        a = Wm[:, :, 1:2 + SZ, 1 + y0:1 + y0 + 17]
                        b = wbt[:, :, 1 + oz:1 + oz + SZ + 1,
                                1 + y0 + oy:1 + y0 + oy + 17]
                        dve.tensor_tensor(out=diff[:], in0=a, in1=b,
                                          op=op.subtract)
                        dve.tensor_tensor(out=diff[:], in0=diff[:],
                                          in1=diff[:], op=op.mult)
                        dv = AP(diff[:].tensor, diff[:].offset,
                                [[KN * (SZ + 1) * 17, 128],
                                 [17, SZ + 1], [1, 17],
                                 [(SZ + 1) * 17, KN]])
                        dve.tensor_reduce(out=drx[d][:], in_=dv,
                                          axis=mybir.AxisListType.X,
                                          op=op.add)

                    # ox == -1 dirs need W at x-1 (wr0-style rotation)
                    nc.sync.dma_start(out=wrb[1:128],
                                      in_=Wd[0:127, :, zc:zc + SZ + 2, :])
                    nc.sync.dma_start(out=wrb[0:1],
                                      in_=Wd[127:128, :, zc:zc + SZ + 2, :])
                    for d in DRD:
                        if OFFS[d][2] == -1:
                            do_dist(d, wrb)
                    for d in DRD:
                        if OFFS[d][2] == 0:
                            do_dist(d, Wm)
                    # ox == +1 dirs need W at x+1 (wr2-style rotation)
                    nc.sync.dma_start(out=wrb[0:127],
                                      in_=Wd[1:128, :, zc:zc + SZ + 2, :])
                    nc.sync.dma_start(out=wrb[127:128],
                                      in_=Wd[0:1, :, zc:zc + SZ + 2, :])
                    for d in DRD:
                        if OFFS[d][2] == 1:
                            do_dist(d, wrb)

                    # logits + rank selection
                    scv = scalem[:, zc:zc + SZ, :]
                    roti = [0]

                    def sel_plane(d):
                        """squared-distance plane for dir d at owned voxels."""
                        if d < 13:
                            oz, oy, ox = OFFS[d]
                            y0 = -1 if oy == 1 else 0
                            return drx[d][:, 0:SZ, -y0:-y0 + 16]
                        dd = 26 - d
                        ozd, oyd, oxd = OFFS[dd]
                        y0d = -1 if oyd == 1 else 0
                        src = drx[dd]
                        if oxd != 0:
                            rt = rot[roti[0] % 2]
                            roti[0] += 1
                            if oxd == -1:
                                nc.sync.dma_start(out=rt[0:127],
                                                  in_=src[1:128])
                                nc.sync.dma_start(out=rt[127:128],
                                                  in_=src[0:1])
                            else:
                                nc.sync.dma_start(out=rt[1:128],
                                                  in_=src[0:127])
                                nc.sync.dma_start(out=rt[0:1],
                                                  in_=src[127:128])
                            src = rt
                        zi = -ozd
                        yi = -oyd - y0d
                        return src[:, zi:zi + SZ, yi:yi + 16]

                    first = True
                    for d in range(27):
                        if d == 13:
                            continue
                        dve.tensor_tensor(out=L[:], in0=sel_plane(d),
                                          in1=scv, op=op.mult)
                        if first:
                            dve.scalar_tensor_tensor(
                                out=es8[:], in0=idxc[:], scalar=float(d),
                                in1=bcast0(L[:], 8), op0=op.is_equal,
                                op1=op.mult)
                            first = False
                        else:
                            dve.scalar_tensor_tensor(
                                out=oc8[:], in0=idxc[:], scalar=float(d),
                                in1=bcast0(L[:], 8), op0=op.is_equal,
                                op1=op.mult)
                            dve.tensor_tensor(out=es8[:], in0=es8[:],
                                              in1=oc8[:], op=op.add)

                    # exp, softmax, output
                    dve.memset(ee9[:, 0], 1.0)
                    act.activation(out=ee9[:, 1:9], in_=es8[:], func=AF.Exp)
                    dve.tensor_tensor(out=s4[:], in0=ee9[:, 0:4],
                                      in1=ee9[:, 4:8], op=op.add)
                    dve.tensor_tensor(out=s4[:, 0:2], in0=s4[:, 0:2],
                                      in1=s4[:, 2:4], op=op.add)
                    dve.tensor_tensor(out=s4[:, 0], in0=s4[:, 0],
                                      in1=s4[:, 1], op=op.add)
                    dve.tensor_tensor(out=s4[:, 0], in0=s4[:, 0],
                                      in1=ee9[:, 8], op=op.add)
                    dve.reciprocal(out=L[:], in_=s4[:, 0])
                    dve.tensor_tensor(out=ee9[:], in0=ee9[:],
                                      in1=bcast0(L[:], KN), op=op.mult)
                    nc.sync.dma_start(out=outd[:, :, zc:zc + SZ, :],
                                      in_=ee9[:])

    nc.compile()
    return nc


# --------------------------------------------------------------------------
# Host side
# --------------------------------------------------------------------------

_CACHED = {}


def _get_nc(ks_value):
    key = float(ks_value)
    if key not in _CACHED:
        _CACHED[key] = build_bass(key)
    return _CACHED[key]


def _shard_inputs(x):
    """x: [H, M, N] f32 -> list of per-core input maps."""
    maps = []
    zext = np.arange(-1, H + 1) % H
    xs = np.arange(N)
    cstv = np.tile((1.0 + 2.0 * np.arange(27) * 2.0**-23)
                   .astype(np.float32).reshape(1, 27, 1), (128, 1, 1))
    for c in range(NCORES):
        ys = (np.arange(YS * c - 2, YS * c + YS + 2)) % M
        slab = x[zext][:, ys, :]                       # [66, 20, 128]
        a = np.empty((128, 3, ZE, YI), dtype=np.float32)
        for r in range(3):
            xrot = (xs + r - 1) % N
            a[:, r] = slab[:, :, xrot].transpose(2, 0, 1)
        maps.append({"xin": np.ascontiguousarray(a), "cst": cstv})
    return maps


def kernel(input, ksigma, k, w):
    from concourse.bass_utils import run_bass_kernel_spmd

    x = np.asarray(input, dtype=np.float32)
    assert x.shape == (H, M, N)
    ks = float(np.asarray(ksigma).reshape(-1)[0])
    assert int(k) == KN and int(w) == 3

    nc = _get_nc(ks)
    in_maps = _shard_inputs(x)
    res = run_bass_kernel_spmd(nc, in_maps, core_ids=list(range(NCORES)))
    full = np.empty((H, M, N, KN), dtype=np.float32)
    for c in range(NCORES):
        oc = res.results[c]["out"]          # [128, KN, H, YS]
        full[:, YS * c:YS * c + YS] = oc.transpose(2, 3, 0, 1)
    return full.reshape(H * M * N, KN)


# revision 17
# speedup vs baseline: 6.2014x; 1.2223x over previous
"""Trainium2 Bass kernel for nn_BuildK (27-neighborhood kNN softmax weights).

v2 design, tuned for the axon backend cost model (per-instruction overhead
dominates): minimize instruction count.

- Sort phase (2 z-chunks of 32): 27 neighbor keys packed as
  trunc(|diff|) * (1 + (2d+sign)*2^-23) in a contiguous [32, FS] tile,
  sorted by a batched Batcher odd-even mergesort (each network level's
  comparator groups run as single strided-AP min/max instructions).
  Decode of the 8 nearest runs batched over ranks. Results (neighbor
  values Wd, neighbor dir indices Id) are staged to DRAM scratch.
- Sigma phase: rowwise unbiased variance -> logit scale.
- Dot phase (2 z-chunks of 32): pairwise feature distances via the exact
  symmetry dist2(v,d) = dist2(v+off_d, 26-d); 13 extended difference
  planes (eps dropped - well within tolerance), per-dir logits selected
  into rank slots with broadcast index-compare ops, single Exp, softmax.
"""

import sys

sys.path.insert(0, "/opt/trn_rl_repo")

import numpy as np

H, M, N = 64, 128, 128
NCORES = 8
YS = M // NCORES          # 16 owned y rows per core
YE = YS + 2               # 18 = sort region (owned + 1 halo each side)
YI = YS + 4               # 20 = input slab y extent (halo 2)
ZE = H + 2                # 66 = z extent with periodic wrap rows
KN = 9
SZ = 32                   # z chunk
FS = SZ * YE              # 576 free elems per sort row

OFFS = [(oz, oy, ox) for oz in (-1, 0, 1) for oy in (-1, 0, 1)
        for ox in (-1, 0, 1)]            # reference enumeration; 13 = center


# --------------------------------------------------------------------------
# Batched Batcher odd-even mergesort schedule for 32 rows.
# Groups: (base, d1, n1, d2, n2, r) -> compare rows (i, i+r),
# i = base + a*d1 + b*d2.  Groups capped at 8 pairs (scratch size).
# --------------------------------------------------------------------------

def _oddeven_comparators(n):
    ops = []

    def merge(lo, m, r):
        step = r * 2
        if step < m:
            merge(lo, m, step)
            merge(lo + r, m, step)
            for i in range(lo + r, lo + m - r, step):
                ops.append((i, i + r))
        else:
            ops.append((lo, lo + r))

    def srt(lo, m):
        if m > 1:
            h = m // 2
            srt(lo, h)
            srt(lo + h, h)
            merge(lo, m, 1)

    srt(0, n)
    return ops


def _grid_decompose(idxs):
    idxs = sorted(idxs)
    grids = []
    rest = idxs
    while rest:
        if len(rest) == 1:
            grids.append((rest[0], 1, 1, 1, 1))
            break
        d1 = rest[1] - rest[0]
        runs = []
        s = rest[0]
        cnt = 1
        for a, b in zip(rest, rest[1:]):
            if b - a == d1:
                cnt += 1
            else:
                runs.append((s, cnt))
                s = b
                cnt = 1
        runs.append((s, cnt))
        n1 = min(c for (_, c) in runs)
        starts = []
        leftover = []
        for (st, c) in runs:
            starts.append(st)
            if c > n1:
                leftover.extend(range(st + n1 * d1, st + c * d1, d1))
        ok2 = True
        d2 = starts[1] - starts[0] if len(starts) > 1 else 1
        for a, b in zip(starts, starts[1:]):
            if b - a != d2:
                ok2 = False
        if ok2:
            grids.append((starts[0], d1, n1, d2, len(starts)))
            rest = sorted(leftover)
        else:
            st, c = runs[0]
            grids.append((st, d1, c, 1, 1))
            rest = sorted(set(rest) - set(range(st, st + c * d1, d1)))
    return grids


def batched_schedule(n=32, maxpairs=8):
    ops = _oddeven_comparators(n)
    level = [0] * n
    lv = []
    for (i, j) in ops:
        l = max(level[i], level[j])
        lv.append((l, i, j))
        level[i] = l + 1
        level[j] = l + 1
    from collections import defaultdict
    bylvr = defaultdict(list)
    for (l, i, j) in lv:
        bylvr[(l, j - i)].append(i)
    sched = []
    for (l, r) in sorted(bylvr.keys()):
        for (base, d1, n1, d2, n2) in _grid_decompose(bylvr[(l, r)]):
            # split so n1*n2 <= maxpairs (split the bigger factor)
            cells = [(base + b * d2, d1, n1) for b in range(n2)]
            cur = []
            cnt = 0
            for (b0, dd1, nn1) in cells:
                while nn1 > 0:
                    take = min(nn1, maxpairs - cnt)
                    cur.append((b0, dd1, take))
                    b0 += take * dd1
                    nn1 -= take
                    cnt += take
                    if cnt == maxpairs:
                        sched.append((cur, r))
                        cur = []
                        cnt = 0
            if cur:
                sched.append((cur, r))
    # each entry: (list of (base, d1, n1) runs, r); regroup runs into
    # (base, d1, n1, d2, n2) when runs are evenly spaced with equal n1
    out = []
    for (runs, r) in sched:
        if len(runs) == 1:
            b, d1, n1 = runs[0]
            out.append((b, d1, n1, 1, 1, r))
        else:
            n1s = set(x[2] for x in runs)
            d1s = set(x[1] for x in runs)
            bs = [x[0] for x in runs]
            gaps = set(b2 - b1 for b1, b2 in zip(bs, bs[1:]))
            if len(n1s) == 1 and len(d1s) == 1 and len(gaps) == 1:
                out.append((bs[0], d1s.pop(), n1s.pop(), gaps.pop(),
                            len(bs), r))
            else:
                for (b, d1, n1) in runs:
                    out.append((b, d1, n1, 1, 1, r))
    return out


SORT_SCHED = batched_schedule(32, maxpairs=16)


# --------------------------------------------------------------------------
# Bass graph
# --------------------------------------------------------------------------

def build_bass(ks_value: float, reps: int = 1):
    from concourse import bacc, mybir
    from concourse import tile
    from concourse.alu_op_type import AluOpType as op
    from concourse.bass_types import AP

    f32 = mybir.dt.float32
    AF = mybir.ActivationFunctionType

    nc = bacc.Bacc("TRN2", target_bir_lowering=False, debug=False,
                   num_devices=NCORES)

    xin = nc.dram_tensor("xin", [128, 3, ZE, YI], f32,
                         kind="ExternalInput").ap()
    cst = nc.dram_tensor("cst", [128, 27, 1], f32, kind="ExternalInput").ap()
    outd = nc.dram_tensor("out", [128, KN, H, YS], f32,
                          kind="ExternalOutput").ap()
    Wd = nc.dram_tensor("Wd", [128, KN, ZE, YE], f32, kind="Internal").ap()
    Id = nc.dram_tensor("Id", [128, 8, H, YS], f32, kind="Internal").ap()

    dve = nc.vector
    act = nc.scalar

    def bcast0(ap, n):
        """Insert a stride-0 dim of length n right after the partition dim."""
        pairs = [list(p) for p in ap.ap]
        newpairs = [pairs[0], [0, n]] + pairs[1:]
        return AP(ap.tensor, ap.offset, newpairs)

    def rows_ap(tile_handle_ap, R, F, base, d1, n1, d2, n2, foff=0, flen=None):
        """AP selecting rows {base + a*d1 + b*d2} of a [128, R, F] tile."""
        if flen is None:
            flen = F
        pairs = [[R * F, 128]]
        if n2 > 1:
            pairs.append([d2 * F, n2])
        if n1 > 1:
            pairs.append([d1 * F, n1])
        pairs.append([1, flen])
        return AP(tile_handle_ap.tensor, tile_handle_ap.offset + base * F + foff,
                  pairs)

    with tile.TileContext(nc) as tc:
      for _rep in range(reps):
        with tc.tile_pool(name="pp", bufs=1) as pp:
            scalem = pp.tile([128, H, YS], f32, tag="scalem")
            cstt = pp.tile([128, 27, 1], f32, tag="cstt")
            nc.sync.dma_start(out=cstt[:], in_=cst[:])

            # ================= sort phase =================
            for zc in (0, SZ):
                with tc.tile_pool(name="sortp", bufs=1) as sp:
                    X3c = sp.tile([128, 3, SZ + 2, YI], f32, tag="X3c")
                    nc.sync.dma_start(out=X3c[:],
                                      in_=xin[:, :, zc:zc + SZ + 2, :])
                    K = sp.tile([128, 32, FS], f32, tag="K")
                    T1 = sp.tile([128, 9, FS], f32, tag="T1")
                    T2 = sp.tile([128, 9, FS], f32, tag="T2")
                    T3 = sp.tile([128, 9, FS], f32, tag="T3")
                    SCR = sp.tile([128, 16, FS], f32, tag="SCR")
                    Wstage = sp.tile([128, KN, SZ, YE], f32, tag="Wstage")

                    cvw = X3c[:, 1, 1:1 + SZ, 1:1 + YE]     # [32,18]

                    dve.memset(K[:, 27:32], 3.0e38)

                    # ---- prep: keys for all 27 dirs ----
                    # grouped diffs straight into K rows, then a 3-pack
                    # ALU pipeline transforms them into packed sort keys
                    for g in range(9):
                        oz, oy = g // 3 - 1, g % 3 - 1
                        vv = X3c[:, :, 1 + oz:1 + oz + SZ,
                                 1 + oy:1 + oy + YE]
                        dve.tensor_tensor(out=K[:, g * 3:g * 3 + 3],
                                          in0=vv, in1=bcast0(cvw, 3),
                                          op=op.subtract)
                    for p in range(3):
                        kp = K[:, 9 * p:9 * p + 9]
                        dve.scalar_tensor_tensor(out=T2[:], in0=kp,
                                                 scalar=-1.0, in1=kp,
                                                 op0=op.mult, op1=op.max)
                        dve.tensor_scalar(out=T3[:], in0=T2[:], scalar1=257.0,
                                          scalar2=None, op0=op.mult)
                        dve.tensor_tensor(out=T2[:], in0=T3[:], in1=T2[:],
                                          op=op.subtract)
                        dve.tensor_tensor(out=T3[:], in0=T3[:], in1=T2[:],
                                          op=op.subtract)       # T3 = hi
                        dve.tensor_scalar(out=T1[:], in0=kp, scalar1=0.0,
                                          scalar2=None, op0=op.is_gt)
                        cr = cstt[:, 9 * p:9 * p + 9, :]
                        dve.scalar_tensor_tensor(
                            out=T1[:], in0=T1[:],
                            scalar=float(np.float32(2.0**-23)),
                            in1=cr.broadcast_to((128, 9, FS)),
                            op0=op.mult, op1=op.add)            # T1 = m
                        dve.tensor_tensor(out=kp, in0=T3[:], in1=T1[:],
                                          op=op.mult)

                    # ---- batched Batcher sort ----
                    for (base, d1, n1, d2, n2, r) in SORT_SCHED:
                        npairs = n1 * n2
                        lo = rows_ap(K[:], 32, FS, base, d1, n1, d2, n2)
                        hi = rows_ap(K[:], 32, FS, base + r, d1, n1, d2, n2)
                        sc = SCR[:, 0:npairs]
                        dve.tensor_tensor(out=sc, in0=lo, in1=hi, op=op.min)
                        dve.tensor_tensor(out=hi, in0=lo, in1=hi, op=op.max)
                        dve.tensor_copy(out=lo, in_=sc)

                    # ---- batched decode of ranks 1..8 ----
                    KS = K[:, 1:9]
                    e1 = T1[:, 0:8]
                    e2 = T2[:, 0:8]
                    e3 = T3[:, 0:8]
                    sc = SCR[:, 0:8]
                    C = float(1.5 * 2.0**23)
                    dve.tensor_scalar(out=e1, in0=KS, scalar1=257.0,
                                      scalar2=None, op0=op.mult)
                    dve.tensor_tensor(out=e2, in0=e1, in1=KS, op=op.subtract)
                    dve.tensor_tensor(out=e3, in0=e1, in1=e2, op=op.subtract)
                    dve.tensor_tensor(out=e1, in0=KS, in1=e3, op=op.subtract)
                    dve.tensor_scalar(out=e2, in0=e3, scalar1=1e-30,
                                      scalar2=None, op0=op.add)
                    dve.reciprocal(out=sc, in_=e2)
                    dve.tensor_tensor(out=e1, in0=e1, in1=sc, op=op.mult)
                    dve.tensor_scalar(out=e1, in0=e1, scalar1=float(2.0**23),
                                      scalar2=C, op0=op.mult, op1=op.add)
                    dve.tensor_scalar(out=e2, in0=e1, scalar1=C, scalar2=None,
                                      op0=op.subtract)          # code
                    dve.tensor_scalar(out=e1, in0=e2, scalar1=-0.5,
                                      scalar2=0.5, op0=op.add, op1=op.mult)
                    dve.tensor_scalar(out=e1, in0=e1, scalar1=C, scalar2=None,
                                      op0=op.add)
                    dve.tensor_scalar(out=e1, in0=e1, scalar1=C, scalar2=None,
                                      op0=op.subtract)          # e1 = delta
                    # idx -> contiguous staging in SCR, then DRAM
                    d_view = AP(T1[:].tensor, T1[:].offset + 1,
                                [[9 * FS, 128], [FS, 8], [YE, SZ], [1, YS]])
                    idq = AP(SCR[:].tensor, SCR[:].offset,
                             [[16 * FS, 128], [SZ * YS, 8], [1, SZ * YS]])
                    dve.tensor_copy(out=idq, in_=d_view)
                    nc.sync.dma_start(out=Id[:, :, zc:zc + SZ, :], in_=idq)
                    # sign & neighbor values
                    dve.scalar_tensor_tensor(out=sc, in0=e1, scalar=-2.0,
                                             in1=e2, op0=op.mult, op1=op.add)
                    dve.tensor_scalar(out=sc, in0=sc, scalar1=2.0,
                                      scalar2=-1.0, op0=op.mult, op1=op.add)
                    dve.tensor_tensor(out=sc, in0=sc, in1=e3, op=op.mult)
                    dve.tensor_tensor(out=Wstage[:, 1:9], in0=sc,
                                      in1=bcast0(cvw, 8), op=op.add)
                    dve.tensor_copy(out=Wstage[:, 0], in_=cvw)
                    nc.sync.dma_start(out=Wd[:, :, 1 + zc:1 + zc + SZ, :],
                                      in_=Wstage[:])
                    # z wrap rows of Wd, straight from the staging tile
                    if zc == 0:
                        nc.sync.dma_start(out=Wd[:, :, ZE - 1:ZE, :],
                                          in_=Wstage[:, :, 0:1, :])
                    else:
                        nc.sync.dma_start(out=Wd[:, :, 0:1, :],
                                          in_=Wstage[:, :, SZ - 1:SZ, :])

            # ================= sigma phase =================
            with tc.tile_pool(name="sigp", bufs=1) as sg:
                Wf = sg.tile([128, KN, ZE, YE], f32, tag="Wf")
                nc.sync.dma_start(out=Wf[:], in_=Wd[:])
                Wo = Wf[:, :, 1:1 + H, 1:1 + YS]      # [9,64,16]
                t4 = sg.tile([128, 4, H, YS], f32, tag="t4")
                sq9 = sg.tile([128, KN, H, YS], f32, tag="sq9")
                S1 = sg.tile([128, H, YS], f32, tag="S1")
                S2 = sg.tile([128, H, YS], f32, tag="S2")
                tv = sg.tile([128, H, YS], f32, tag="tv")
                dve.tensor_tensor(out=t4[:], in0=Wf[:, 0:4, 1:1 + H, 1:1 + YS],
                                  in1=Wf[:, 4:8, 1:1 + H, 1:1 + YS], op=op.add)
                dve.tensor_tensor(out=t4[:, 0:2], in0=t4[:, 0:2],
                                  in1=t4[:, 2:4], op=op.add)
                dve.tensor_tensor(out=t4[:, 0], in0=t4[:, 0], in1=t4[:, 1],
                                  op=op.add)
                dve.tensor_tensor(out=S1[:], in0=t4[:, 0],
                                  in1=Wf[:, 8, 1:1 + H, 1:1 + YS], op=op.add)
                dve.tensor_tensor(out=sq9[:], in0=Wo, in1=Wo, op=op.mult)
                dve.tensor_tensor(out=t4[:], in0=sq9[:, 0:4], in1=sq9[:, 4:8],
                                  op=op.add)
                dve.tensor_tensor(out=t4[:, 0:2], in0=t4[:, 0:2],
                                  in1=t4[:, 2:4], op=op.add)
                dve.tensor_tensor(out=t4[:, 0], in0=t4[:, 0], in1=t4[:, 1],
                                  op=op.add)
                dve.tensor_tensor(out=S2[:], in0=t4[:, 0], in1=sq9[:, 8],
                                  op=op.add)
                dve.tensor_tensor(out=tv[:], in0=S1[:], in1=S1[:], op=op.mult)
                dve.scalar_tensor_tensor(out=tv[:], in0=tv[:],
                                         scalar=-1.0 / 9.0, in1=S2[:],
                                         op0=op.mult, op1=op.add)  # tvar
                dve.tensor_scalar(out=S1[:], in0=tv[:], scalar1=0.0,
                                  scalar2=None, op0=op.is_equal)
                dve.tensor_tensor(out=S1[:], in0=S1[:], in1=tv[:], op=op.add)
                dve.reciprocal(out=S2[:], in_=S1[:])
                dve.tensor_scalar(out=S2[:], in0=S2[:],
                                  scalar1=-4.0 / (ks_value * ks_value),
                                  scalar2=None, op0=op.mult)
                dve.tensor_scalar(out=S1[:], in0=tv[:], scalar1=0.0,
                                  scalar2=None, op0=op.not_equal)
                dve.tensor_tensor(out=scalem[:], in0=S2[:], in1=S1[:],
                                  op=op.mult)

            # ================= dot phase =================
            DXM = [0, 3, 6, 9, 12]     # ox == -1
            DX0 = [1, 4, 7, 10]        # ox == 0
            DXP = [2, 5, 8, 11]        # ox == +1
            PL = (SZ + 1) * 17         # 561 elems per distance plane
            for zc in (0, SZ):
                with tc.tile_pool(name="dotp", bufs=1) as dp:
                    Wm = dp.tile([128, KN, SZ + 2, YE], f32, tag="Wm")
                    nc.sync.dma_start(out=Wm[:],
                                      in_=Wd[:, :, zc:zc + SZ + 2, :])
                    wrb = dp.tile([128, KN, SZ + 2, YE], f32, tag="wrb")
                    idxc = dp.tile([128, 8, SZ * YS], f32, tag="idxc")
                    nc.sync.dma_start(out=idxc[:],
                                      in_=Id[:, :, zc:zc + SZ, :])
                    diff = dp.tile([128, KN, SZ + 1, 17], f32, tag="diff")
                    drxm = dp.tile([128, 5, PL], f32, tag="drxm")
                    drx0 = dp.tile([128, 4, PL], f32, tag="drx0")
                    drxp = dp.tile([128, 4, PL], f32, tag="drxp")
                    rotm = dp.tile([128, 5, PL], f32, tag="rotm")
                    rotp = dp.tile([128, 4, PL], f32, tag="rotp")
                    L = dp.tile([128, SZ * YS], f32, tag="L")
                    es8 = dp.tile([128, 8, SZ * YS], f32, tag="es8")
                    oc8 = dp.tile([128, 8, SZ * YS], f32, tag="oc8")
                    ee9 = dp.tile([128, KN, SZ * YS], f32, tag="ee9")
                    s4 = dp.tile([128, 4, SZ * YS], f32, tag="s4")

                    CAT = {}
                    for i, d in enumerate(DXM):
                        CAT[d] = (drxm, 5, i)
                    for i, d in enumerate(DX0):
                        CAT[d] = (drx0, 4, i)
                    for i, d in enumerate(DXP):
                        CAT[d] = (drxp, 4, i)
                    ROT = {d: (rotm, 5, i) for i, d in enumerate(DXM)}
                    ROT.update({d: (rotp, 4, i) for i, d in enumerate(DXP)})

                    # 13 extended squared-distance planes, grouped by ox so
                    # one rotated slab buffer suffices
                    def do_dist(d, wbt):
                        oz, oy, ox = OFFS[d]
                        y0 = -1 if oy == 1 else 0
                        a = Wm[:, :, 1:2 + SZ, 1 + y0:1 + y0 + 17]
                        b = wbt[:, :, 1 + oz:1 + oz + SZ + 1,
                                1 + y0 + oy:1 + y0 + oy + 17]
                        dve.tensor_tensor(out=diff[:], in0=a, in1=b,
                                          op=op.subtract)
                        dve.tensor_tensor(out=diff[:], in0=diff[:],
                                          in1=diff[:], op=op.mult)
                        dv = AP(diff[:].tensor, diff[:].offset,
                                [[KN * PL, 128],
                                 [17, SZ + 1], [1, 17],
                                 [PL, KN]])
                        cat, ncat, k = CAT[d]
                        dve.tensor_reduce(out=cat[:, k], in_=dv,
                                          axis=mybir.AxisListType.X,
                                          op=op.add)

                    # ox == -1 dirs need W at x-1 (wr0-style rotation)
                    nc.sync.dma_start(out=wrb[1:128],
                                      in_=Wd[0:127, :, zc:zc + SZ + 2, :])
                    nc.sync.dma_start(out=wrb[0:1],
                                      in_=Wd[127:128, :, zc:zc + SZ + 2, :])
                    for d in DXM:
                        do_dist(d, wrb)
                    for d in DX0:
                        do_dist(d, Wm)
                    # ox == +1 dirs need W at x+1 (wr2-style rotation)
                    nc.sync.dma_start(out=wrb[0:127],
                                      in_=Wd[1:128, :, zc:zc + SZ + 2, :])
                    nc.sync.dma_start(out=wrb[127:128],
                                      in_=Wd[0:1, :, zc:zc + SZ + 2, :])
                    for d in DXP:
                        do_dist(d, wrb)

                    # rotate the ox != 0 plane groups across partitions once
                    nc.sync.dma_start(out=rotm[0:127], in_=drxm[1:128])
                    nc.sync.dma_start(out=rotm[127:128], in_=drxm[0:1])
                    nc.sync.dma_start(out=rotp[1:128], in_=drxp[0:127])
                    nc.sync.dma_start(out=rotp[0:1], in_=drxp[127:128])

                    # logits + rank selection
                    scv = scalem[:, zc:zc + SZ, :]

                    def pwin(cat, ncat, k, z0, y0v):
                        """[SZ,16] window at (z0, y0v) of plane k."""
                        base = cat[:]
                        return AP(base.tensor,
                                  base.offset + k * PL + z0 * 17 + y0v,
                                  [[ncat * PL, 128], [17, SZ], [1, 16]])

                    def sel_plane(d):
                        """squared-distance plane for dir d at owned voxels."""
                        if d < 13:
                            oz, oy, ox = OFFS[d]
                            y0 = -1 if oy == 1 else 0
                            cat, ncat, k = CAT[d]
                            return pwin(cat, ncat, k, 0, -y0)
                        dd = 26 - d
                        ozd, oyd, oxd = OFFS[dd]
                        y0d = -1 if oyd == 1 else 0
                        cat, ncat, k = ROT[dd] if oxd != 0 else CAT[dd]
                        return pwin(cat, ncat, k, -ozd, -oyd - y0d)

                    first = True
                    for d in range(27):
                        if d == 13:
                            continue
                        dve.tensor_tensor(out=L[:], in0=sel_plane(d),
                                          in1=scv, op=op.mult)
                        if first:
                            dve.scalar_tensor_tensor(
                                out=es8[:], in0=idxc[:], scalar=float(d),
                                in1=bcast0(L[:], 8), op0=op.is_equal,
                                op1=op.mult)
                            first = False
                        else:
                            dve.scalar_tensor_tensor(
                                out=oc8[:], in0=idxc[:], scalar=float(d),
                                in1=bcast0(L[:], 8), op0=op.is_equal,
                                op1=op.mult)
                            dve.tensor_tensor(out=es8[:], in0=es8[:],
                                              in1=oc8[:], op=op.add)

                    # exp, softmax, output
                    dve.memset(ee9[:, 0], 1.0)
                    act.activation(out=ee9[:, 1:9], in_=es8[:], func=AF.Exp)
                    dve.tensor_tensor(out=s4[:], in0=ee9[:, 0:4],
                                      in1=ee9[:, 4:8], op=op.add)
                    dve.tensor_tensor(out=s4[:, 0:2], in0=s4[:, 0:2],
                                      in1=s4[:, 2:4], op=op.add)
                    dve.tensor_tensor(out=s4[:, 0], in0=s4[:, 0],
                                      in1=s4[:, 1], op=op.add)
                    dve.tensor_tensor(out=s4[:, 0], in0=s4[:, 0],
                                      in1=ee9[:, 8], op=op.add)
                    dve.reciprocal(out=L[:], in_=s4[:, 0])
                    dve.tensor_tensor(out=ee9[:], in0=ee9[:],
                                      in1=bcast0(L[:], KN), op=op.mult)
                    nc.sync.dma_start(out=outd[:, :, zc:zc + SZ, :],
                                      in_=ee9[:])

    nc.compile()
    return nc


# --------------------------------------------------------------------------
# Host side
# --------------------------------------------------------------------------

_CACHED = {}


def _get_nc(ks_value):
    key = float(ks_value)
    if key not in _CACHED:
        _CACHED[key] = build_bass(key)
    return _CACHED[key]


def _shard_inputs(x):
    """x: [H, M, N] f32 -> list of per-core input maps."""
    maps = []
    zext = np.arange(-1, H + 1) % H
    xs = np.arange(N)
    cstv = np.tile((1.0 + 2.0 * np.arange(27) * 2.0**-23)
                   .astype(np.float32).reshape(1, 27, 1), (128, 1, 1))
    for c in range(NCORES):
        ys = (np.arange(YS * c - 2, YS * c + YS + 2)) % M
        slab = x[zext][:, ys, :]                       # [66, 20, 128]
        a = np.empty((128, 3, ZE, YI), dtype=np.float32)
        for r in range(3):
            xrot = (xs + r - 1) % N
            a[:, r] = slab[:, :, xrot].transpose(2, 0, 1)
        maps.append({"xin": np.ascontiguousarray(a), "cst": cstv})
    return maps


def kernel(input, ksigma, k, w):
    from concourse.bass_utils import run_bass_kernel_spmd

    x = np.asarray(input, dtype=np.float32)
    assert x.shape == (H, M, N)
    ks = float(np.asarray(ksigma).reshape(-1)[0])
    assert int(k) == KN and int(w) == 3

    nc = _get_nc(ks)
    in_maps = _shard_inputs(x)
    res = run_bass_kernel_spmd(nc, in_maps, core_ids=list(range(NCORES)))
    full = np.empty((H, M, N, KN), dtype=np.float32)
    for c in range(NCORES):
        oc = res.results[c]["out"]          # [128, KN, H, YS]
        full[:, YS * c:YS * c + YS] = oc.transpose(2, 3, 0, 1)
    return full.reshape(H * M * N, KN)


# revision 23
# speedup vs baseline: 16.2844x; 2.6259x over previous
"""Trainium2 Bass kernel for nn_BuildK (27-neighborhood kNN softmax weights).

v2 design, tuned for the axon backend cost model (per-instruction overhead
dominates): minimize instruction count.

- Sort phase (2 z-chunks of 32): 27 neighbor keys packed as
  trunc(|diff|) * (1 + (2d+sign)*2^-23) in a contiguous [32, FS] tile,
  sorted by a batched Batcher odd-even mergesort (each network level's
  comparator groups run as single strided-AP min/max instructions).
  Decode of the 8 nearest runs batched over ranks. Results (neighbor
  values Wd, neighbor dir indices Id) are staged to DRAM scratch.
- Sigma phase: rowwise unbiased variance -> logit scale.
- Dot phase (2 z-chunks of 32): pairwise feature distances via the exact
  symmetry dist2(v,d) = dist2(v+off_d, 26-d); 13 extended difference
  planes (eps dropped - well within tolerance), per-dir logits selected
  into rank slots with broadcast index-compare ops, single Exp, softmax.
"""

import sys

sys.path.insert(0, "/opt/trn_rl_repo")

import numpy as np

H, M, N = 64, 128, 128
NCORES = 8
YS = M // NCORES          # 16 owned y rows per core
YE = YS + 2               # 18 = sort region (owned + 1 halo each side)
YI = YS + 4               # 20 = input slab y extent (halo 2)
ZE = H + 2                # 66 = z extent with periodic wrap rows
KN = 9
SZ = 32                   # z chunk
FS = SZ * YE              # 576 free elems per sort row

OFFS = [(oz, oy, ox) for oz in (-1, 0, 1) for oy in (-1, 0, 1)
        for ox in (-1, 0, 1)]            # reference enumeration; 13 = center


# --------------------------------------------------------------------------
# Batched Batcher odd-even mergesort schedule for 32 rows.
# Groups: (base, d1, n1, d2, n2, r) -> compare rows (i, i+r),
# i = base + a*d1 + b*d2.  Groups capped at 8 pairs (scratch size).
# --------------------------------------------------------------------------

def _oddeven_comparators(n):
    ops = []

    def merge(lo, m, r):
        step = r * 2
        if step < m:
            merge(lo, m, step)
            merge(lo + r, m, step)
            for i in range(lo + r, lo + m - r, step):
                ops.append((i, i + r))
        else:
            ops.append((lo, lo + r))

    def srt(lo, m):
        if m > 1:
            h = m // 2
            srt(lo, h)
            srt(lo + h, h)
            merge(lo, m, 1)

    srt(0, n)
    return ops


def _grid_decompose(idxs):
    idxs = sorted(idxs)
    grids = []
    rest = idxs
    while rest:
        if len(rest) == 1:
            grids.append((rest[0], 1, 1, 1, 1))
            break
        d1 = rest[1] - rest[0]
        runs = []
        s = rest[0]
        cnt = 1
        for a, b in zip(rest, rest[1:]):
            if b - a == d1:
                cnt += 1
            else:
                runs.append((s, cnt))
                s = b
                cnt = 1
        runs.append((s, cnt))
        n1 = min(c for (_, c) in runs)
        starts = []
        leftover = []
        for (st, c) in runs:
            starts.append(st)
            if c > n1:
                leftover.extend(range(st + n1 * d1, st + c * d1, d1))
        ok2 = True
        d2 = starts[1] - starts[0] if len(starts) > 1 else 1
        for a, b in zip(starts, starts[1:]):
            if b - a != d2:
                ok2 = False
        if ok2:
            grids.append((starts[0], d1, n1, d2, len(starts)))
            rest = sorted(leftover)
        else:
            st, c = runs[0]
            grids.append((st, d1, c, 1, 1))
            rest = sorted(set(rest) - set(range(st, st + c * d1, d1)))
    return grids


def batched_schedule(n=32, maxpairs=8):
    ops = _oddeven_comparators(n)
    level = [0] * n
    lv = []
    for (i, j) in ops:
        l = max(level[i], level[j])
        lv.append((l, i, j))
        level[i] = l + 1
        level[j] = l + 1
    from collections import defaultdict
    bylvr = defaultdict(list)
    for (l, i, j) in lv:
        bylvr[(l, j - i)].append(i)
    sched = []
    for (l, r) in sorted(bylvr.keys()):
        for (base, d1, n1, d2, n2) in _grid_decompose(bylvr[(l, r)]):
            # split so n1*n2 <= maxpairs (split the bigger factor)
            cells = [(base + b * d2, d1, n1) for b in range(n2)]
            cur = []
            cnt = 0
            for (b0, dd1, nn1) in cells:
                while nn1 > 0:
                    take = min(nn1, maxpairs - cnt)
                    cur.append((b0, dd1, take))
                    b0 += take * dd1
                    nn1 -= take
                    cnt += take
                    if cnt == maxpairs:
                        sched.append((cur, r))
                        cur = []
                        cnt = 0
            if cur:
                sched.append((cur, r))
    # each entry: (list of (base, d1, n1) runs, r); regroup runs into
    # (base, d1, n1, d2, n2) when runs are evenly spaced with equal n1
    out = []
    for (runs, r) in sched:
        if len(runs) == 1:
            b, d1, n1 = runs[0]
            out.append((b, d1, n1, 1, 1, r))
        else:
            n1s = set(x[2] for x in runs)
            d1s = set(x[1] for x in runs)
            bs = [x[0] for x in runs]
            gaps = set(b2 - b1 for b1, b2 in zip(bs, bs[1:]))
            if len(n1s) == 1 and len(d1s) == 1 and len(gaps) == 1:
                out.append((bs[0], d1s.pop(), n1s.pop(), gaps.pop(),
                            len(bs), r))
            else:
                for (b, d1, n1) in runs:
                    out.append((b, d1, n1, 1, 1, r))
    return out


SORT_SCHED = batched_schedule(32, maxpairs=16)


# --------------------------------------------------------------------------
# Bass graph
# --------------------------------------------------------------------------

def build_bass(ks_value: float, reps: int = 1):
    from concourse import bacc, mybir
    from concourse import tile
    from concourse.alu_op_type import AluOpType as op
    from concourse.bass_types import AP

    f32 = mybir.dt.float32
    AF = mybir.ActivationFunctionType

    nc = bacc.Bacc("TRN2", target_bir_lowering=False, debug=False,
                   num_devices=NCORES)

    xin = nc.dram_tensor("xin", [128, 3, ZE, YI], f32,
                         kind="ExternalInput").ap()
    cst = nc.dram_tensor("cst", [128, 27, 1], f32, kind="ExternalInput").ap()
    outd = nc.dram_tensor("out", [128, KN, H, YS], f32,
                          kind="ExternalOutput").ap()
    Wd = nc.dram_tensor("Wd", [128, KN, ZE, YE], f32, kind="Internal").ap()
    Id = nc.dram_tensor("Id", [128, 8, H, YS], f32, kind="Internal").ap()

    dve = nc.vector
    act = nc.scalar

    def bcast0(ap, n):
        """Insert a stride-0 dim of length n right after the partition dim."""
        pairs = [list(p) for p in ap.ap]
        newpairs = [pairs[0], [0, n]] + pairs[1:]
        return AP(ap.tensor, ap.offset, newpairs)

    def rows_ap(tile_handle_ap, R, F, base, d1, n1, d2, n2, foff=0, flen=None):
        """AP selecting rows {base + a*d1 + b*d2} of a [128, R, F] tile."""
        if flen is None:
            flen = F
        pairs = [[R * F, 128]]
        if n2 > 1:
            pairs.append([d2 * F, n2])
        if n1 > 1:
            pairs.append([d1 * F, n1])
        pairs.append([1, flen])
        return AP(tile_handle_ap.tensor, tile_handle_ap.offset + base * F + foff,
                  pairs)

    with tile.TileContext(nc) as tc:
      for _rep in range(reps):
        with tc.tile_pool(name="pp", bufs=1) as pp:
            scalem = pp.tile([128, H, YS], f32, tag="scalem")
            cstt = pp.tile([128, 27, 1], f32, tag="cstt")
            nc.sync.dma_start(out=cstt[:], in_=cst[:])

            # ================= sort phase =================
            for zc in (0, SZ):
                with tc.tile_pool(name="sortp", bufs=1) as sp:
                    X3c = sp.tile([128, 3, SZ + 2, YI], f32, tag="X3c")
                    nc.sync.dma_start(out=X3c[:],
                                      in_=xin[:, :, zc:zc + SZ + 2, :])
                    K = sp.tile([128, 32, FS], f32, tag="K")
                    T2 = sp.tile([128, 14, FS], f32, tag="T2")
                    T3 = sp.tile([128, 14, FS], f32, tag="T3")
                    SCR = sp.tile([128, 16, FS], f32, tag="SCR")
                    Wstage = sp.tile([128, KN, SZ, YE], f32, tag="Wstage")

                    cvw = X3c[:, 1, 1:1 + SZ, 1:1 + YE]     # [32,18]

                    dve.memset(K[:, 27:32], 3.0e38)

                    # ---- prep: keys for all 27 dirs ----
                    # grouped diffs straight into K rows, then a 3-pack
                    # ALU pipeline transforms them into packed sort keys
                    for g in range(9):
                        oz, oy = g // 3 - 1, g % 3 - 1
                        vv = X3c[:, :, 1 + oz:1 + oz + SZ,
                                 1 + oy:1 + oy + YE]
                        dve.tensor_tensor(out=K[:, g * 3:g * 3 + 3],
                                          in0=vv, in1=bcast0(cvw, 3),
                                          op=op.subtract)
                    for (r0, nr) in ((0, 14), (14, 13)):
                        kp = K[:, r0:r0 + nr]
                        t1 = SCR[:, 0:nr]
                        t2 = T2[:, 0:nr]
                        t3 = T3[:, 0:nr]
                        dve.scalar_tensor_tensor(out=t2, in0=kp,
                                                 scalar=-1.0, in1=kp,
                                                 op0=op.mult, op1=op.max)
                        dve.tensor_scalar(out=t3, in0=t2, scalar1=257.0,
                                          scalar2=None, op0=op.mult)
                        dve.tensor_tensor(out=t2, in0=t3, in1=t2,
                                          op=op.subtract)
                        dve.tensor_tensor(out=t3, in0=t3, in1=t2,
                                          op=op.subtract)       # t3 = hi
                        dve.tensor_scalar(out=t1, in0=kp, scalar1=0.0,
                                          scalar2=None, op0=op.is_gt)
                        cr = cstt[:, r0:r0 + nr, :]
                        dve.scalar_tensor_tensor(
                            out=t1, in0=t1,
                            scalar=float(np.float32(2.0**-23)),
                            in1=cr.broadcast_to((128, nr, FS)),
                            op0=op.mult, op1=op.add)            # t1 = m
                        dve.tensor_tensor(out=kp, in0=t3, in1=t1,
                                          op=op.mult)

                    # ---- batched Batcher sort ----
                    for (base, d1, n1, d2, n2, r) in SORT_SCHED:
                        npairs = n1 * n2
                        lo = rows_ap(K[:], 32, FS, base, d1, n1, d2, n2)
                        hi = rows_ap(K[:], 32, FS, base + r, d1, n1, d2, n2)
                        sc = SCR[:, 0:npairs]
                        dve.tensor_tensor(out=sc, in0=lo, in1=hi, op=op.min)
                        dve.tensor_tensor(out=hi, in0=lo, in1=hi, op=op.max)
                        dve.tensor_copy(out=lo, in_=sc)

                    # ---- batched decode of ranks 1..8 ----
                    KS = K[:, 1:9]
                    e1 = SCR[:, 0:8]
                    e2 = T2[:, 0:8]
                    e3 = T3[:, 0:8]
                    sc = SCR[:, 8:16]
                    C = float(1.5 * 2.0**23)
                    dve.tensor_scalar(out=e1, in0=KS, scalar1=257.0,
                                      scalar2=None, op0=op.mult)
                    dve.tensor_tensor(out=e2, in0=e1, in1=KS, op=op.subtract)
                    dve.tensor_tensor(out=e3, in0=e1, in1=e2, op=op.subtract)
                    dve.tensor_tensor(out=e1, in0=KS, in1=e3, op=op.subtract)
                    dve.tensor_scalar(out=e2, in0=e3, scalar1=1e-30,
                                      scalar2=None, op0=op.add)
                    dve.reciprocal(out=sc, in_=e2)
                    dve.tensor_tensor(out=e1, in0=e1, in1=sc, op=op.mult)
                    dve.tensor_scalar(out=e1, in0=e1, scalar1=float(2.0**23),
                                      scalar2=C, op0=op.mult, op1=op.add)
                    dve.tensor_scalar(out=e2, in0=e1, scalar1=C, scalar2=None,
                                      op0=op.subtract)          # code
                    dve.tensor_scalar(out=e1, in0=e2, scalar1=-0.5,
                                      scalar2=0.5, op0=op.add, op1=op.mult)
                    dve.tensor_scalar(out=e1, in0=e1, scalar1=C, scalar2=None,
                                      op0=op.add)
                    dve.tensor_scalar(out=e1, in0=e1, scalar1=C, scalar2=None,
                                      op0=op.subtract)          # e1 = delta
                    # idx -> contiguous staging in SCR, then DRAM
                    d_view = AP(SCR[:].tensor, SCR[:].offset + 1,
                                [[16 * FS, 128], [FS, 8], [YE, SZ], [1, YS]])
                    idq = AP(SCR[:].tensor, SCR[:].offset + 8 * FS,
                             [[16 * FS, 128], [SZ * YS, 8], [1, SZ * YS]])
                    dve.tensor_copy(out=idq, in_=d_view)
                    nc.sync.dma_start(out=Id[:, :, zc:zc + SZ, :], in_=idq)
                    # sign & neighbor values
                    dve.scalar_tensor_tensor(out=sc, in0=e1, scalar=-2.0,
                                             in1=e2, op0=op.mult, op1=op.add)
                    dve.tensor_scalar(out=sc, in0=sc, scalar1=2.0,
                                      scalar2=-1.0, op0=op.mult, op1=op.add)
                    dve.tensor_tensor(out=sc, in0=sc, in1=e3, op=op.mult)
                    dve.tensor_tensor(out=Wstage[:, 1:9], in0=sc,
                                      in1=bcast0(cvw, 8), op=op.add)
                    dve.tensor_copy(out=Wstage[:, 0], in_=cvw)
                    nc.sync.dma_start(out=Wd[:, :, 1 + zc:1 + zc + SZ, :],
                                      in_=Wstage[:])
                    # z wrap rows of Wd, straight from the staging tile
                    if zc == 0:
                        nc.sync.dma_start(out=Wd[:, :, ZE - 1:ZE, :],
                                          in_=Wstage[:, :, 0:1, :])
                    else:
                        nc.sync.dma_start(out=Wd[:, :, 0:1, :],
                                          in_=Wstage[:, :, SZ - 1:SZ, :])

            # ================= sigma phase =================
            with tc.tile_pool(name="sigp", bufs=1) as sg:
                Wf = sg.tile([128, KN, ZE, YE], f32, tag="Wf")
                nc.sync.dma_start(out=Wf[:], in_=Wd[:])
                Wo = Wf[:, :, 1:1 + H, 1:1 + YS]      # [9,64,16]
                sq9 = sg.tile([128, KN, H * YS], f32, tag="sq9")
                S1 = sg.tile([128, H, YS], f32, tag="S1")
                S2 = sg.tile([128, H, YS], f32, tag="S2")
                tv = sg.tile([128, H, YS], f32, tag="tv")
                wfa = Wf[:]
                wro = AP(wfa.tensor, wfa.offset + YE + 1,
                         [[KN * ZE * YE, 128], [YE, H], [1, YS],
                          [ZE * YE, KN]])
                dve.tensor_reduce(out=S1[:], in_=wro,
                                  axis=mybir.AxisListType.X, op=op.add)
                dve.tensor_tensor(out=sq9[:], in0=Wo, in1=Wo, op=op.mult)
                sqa = sq9[:]
                sqro = AP(sqa.tensor, sqa.offset,
                          [[KN * H * YS, 128], [1, H * YS], [H * YS, KN]])
                dve.tensor_reduce(out=S2[:], in_=sqro,
                                  axis=mybir.AxisListType.X, op=op.add)
                dve.tensor_tensor(out=tv[:], in0=S1[:], in1=S1[:], op=op.mult)
                dve.scalar_tensor_tensor(out=tv[:], in0=tv[:],
                                         scalar=-1.0 / 9.0, in1=S2[:],
                                         op0=op.mult, op1=op.add)  # tvar
                dve.tensor_scalar(out=S1[:], in0=tv[:], scalar1=0.0,
                                  scalar2=None, op0=op.is_equal)
                dve.tensor_tensor(out=S1[:], in0=S1[:], in1=tv[:], op=op.add)
                dve.reciprocal(out=S2[:], in_=S1[:])
                dve.tensor_scalar(out=S2[:], in0=S2[:],
                                  scalar1=-4.0 / (ks_value * ks_value),
                                  scalar2=None, op0=op.mult)
                dve.tensor_scalar(out=S1[:], in0=tv[:], scalar1=0.0,
                                  scalar2=None, op0=op.not_equal)
                dve.tensor_tensor(out=scalem[:], in0=S2[:], in1=S1[:],
                                  op=op.mult)

            # ================= dot phase =================
            DXM = [0, 3, 6, 9, 12]     # ox == -1
            DX0 = [1, 4, 7, 10]        # ox == 0
            DXP = [2, 5, 8, 11]        # ox == +1
            PL = (SZ + 1) * 17         # 561 elems per distance plane
            for zc in (0, SZ):
                with tc.tile_pool(name="dotp", bufs=1) as dp:
                    Wm = dp.tile([128, KN, SZ + 2, YE], f32, tag="Wm")
                    nc.sync.dma_start(out=Wm[:],
                                      in_=Wd[:, :, zc:zc + SZ + 2, :])
                    wrb = dp.tile([128, KN, SZ + 2, YE], f32, tag="wrb")
                    idxc = dp.tile([128, 8, SZ * YS], f32, tag="idxc")
                    nc.sync.dma_start(out=idxc[:],
                                      in_=Id[:, :, zc:zc + SZ, :])
                    diff = dp.tile([128, KN, SZ + 1, 17], f32, tag="diff")
                    drxm = dp.tile([128, 5, PL], f32, tag="drxm")
                    drx0 = dp.tile([128, 4, PL], f32, tag="drx0")
                    drxp = dp.tile([128, 4, PL], f32, tag="drxp")
                    rotm = dp.tile([128, 5, PL], f32, tag="rotm")
                    rotp = dp.tile([128, 4, PL], f32, tag="rotp")
                    L = dp.tile([128, SZ * YS], f32, tag="L")
                    es8 = dp.tile([128, 8, SZ * YS], f32, tag="es8")
                    oc8 = dp.tile([128, 8, SZ * YS], f32, tag="oc8")
                    ee9 = dp.tile([128, KN, SZ * YS], f32, tag="ee9")
                    s4 = dp.tile([128, 4, SZ * YS], f32, tag="s4")

                    CAT = {}
                    for i, d in enumerate(DXM):
                        CAT[d] = (drxm, 5, i)
                    for i, d in enumerate(DX0):
                        CAT[d] = (drx0, 4, i)
                    for i, d in enumerate(DXP):
                        CAT[d] = (drxp, 4, i)
                    ROT = {d: (rotm, 5, i) for i, d in enumerate(DXM)}
                    ROT.update({d: (rotp, 4, i) for i, d in enumerate(DXP)})

                    # 13 extended squared-distance planes, grouped by ox so
                    # one rotated slab buffer suffices
                    def do_dist(d, wbt):
                        oz, oy, ox = OFFS[d]
                        y0 = -1 if oy == 1 else 0
                        a = Wm[:, :, 1:2 + SZ, 1 + y0:1 + y0 + 17]
                        b = wbt[:, :, 1 + oz:1 + oz + SZ + 1,
                                1 + y0 + oy:1 + y0 + oy + 17]
                        dve.tensor_tensor(out=diff[:], in0=a, in1=b,
                                          op=op.subtract)
                        dve.tensor_tensor(out=diff[:], in0=diff[:],
                                          in1=diff[:], op=op.mult)
                        dv = AP(diff[:].tensor, diff[:].offset,
                                [[KN * PL, 128],
                                 [17, SZ + 1], [1, 17],
                                 [PL, KN]])
                        cat, ncat, k = CAT[d]
                        dve.tensor_reduce(out=cat[:, k], in_=dv,
                                          axis=mybir.AxisListType.X,
                                          op=op.add)

                    # ox == -1 dirs need W at x-1 (wr0-style rotation)
                    nc.sync.dma_start(out=wrb[1:128],
                                      in_=Wd[0:127, :, zc:zc + SZ + 2, :])
                    nc.sync.dma_start(out=wrb[0:1],
                                      in_=Wd[127:128, :, zc:zc + SZ + 2, :])
                    for d in DXM:
                        do_dist(d, wrb)
                    for d in DX0:
                        do_dist(d, Wm)
                    # ox == +1 dirs need W at x+1 (wr2-style rotation)
                    nc.sync.dma_start(out=wrb[0:127],
                                      in_=Wd[1:128, :, zc:zc + SZ + 2, :])
                    nc.sync.dma_start(out=wrb[127:128],
                                      in_=Wd[0:1, :, zc:zc + SZ + 2, :])
                    for d in DXP:
                        do_dist(d, wrb)

                    # rotate the ox != 0 plane groups across partitions once
                    nc.sync.dma_start(out=rotm[0:127], in_=drxm[1:128])
                    nc.sync.dma_start(out=rotm[127:128], in_=drxm[0:1])
                    nc.sync.dma_start(out=rotp[1:128], in_=drxp[0:127])
                    nc.sync.dma_start(out=rotp[0:1], in_=drxp[127:128])

                    # logits + rank selection
                    scv = scalem[:, zc:zc + SZ, :]

                    def pwin(cat, ncat, k, z0, y0v):
                        """[SZ,16] window at (z0, y0v) of plane k."""
                        base = cat[:]
                        return AP(base.tensor,
                                  base.offset + k * PL + z0 * 17 + y0v,
                                  [[ncat * PL, 128], [17, SZ], [1, 16]])

                    def sel_plane(d):
                        """squared-distance plane for dir d at owned voxels."""
                        if d < 13:
                            oz, oy, ox = OFFS[d]
                            y0 = -1 if oy == 1 else 0
                            cat, ncat, k = CAT[d]
                            return pwin(cat, ncat, k, 0, -y0)
                        dd = 26 - d
                        ozd, oyd, oxd = OFFS[dd]
                        y0d = -1 if oyd == 1 else 0
                        cat, ncat, k = ROT[dd] if oxd != 0 else CAT[dd]
                        return pwin(cat, ncat, k, -ozd, -oyd - y0d)

                    first = True
                    for d in range(27):
                        if d == 13:
                            continue
                        dve.tensor_tensor(out=L[:], in0=sel_plane(d),
                                          in1=scv, op=op.mult)
                        if first:
                            dve.scalar_tensor_tensor(
                                out=es8[:], in0=idxc[:], scalar=float(d),
                                in1=bcast0(L[:], 8), op0=op.is_equal,
                                op1=op.mult)
                            first = False
                        else:
                            dve.scalar_tensor_tensor(
                                out=oc8[:], in0=idxc[:], scalar=float(d),
                                in1=bcast0(L[:], 8), op0=op.is_equal,
                                op1=op.mult)
                            dve.tensor_tensor(out=es8[:], in0=es8[:],
                                              in1=oc8[:], op=op.add)

                    # exp, softmax, output
                    dve.memset(ee9[:, 0], 1.0)
                    act.activation(out=ee9[:, 1:9], in_=es8[:], func=AF.Exp)
                    dve.tensor_tensor(out=s4[:], in0=ee9[:, 0:4],
                                      in1=ee9[:, 4:8], op=op.add)
                    dve.tensor_tensor(out=s4[:, 0:2], in0=s4[:, 0:2],
                                      in1=s4[:, 2:4], op=op.add)
                    dve.tensor_tensor(out=s4[:, 0], in0=s4[:, 0],
                                      in1=s4[:, 1], op=op.add)
                    dve.tensor_tensor(out=s4[:, 0], in0=s4[:, 0],
                                      in1=ee9[:, 8], op=op.add)
                    dve.reciprocal(out=L[:], in_=s4[:, 0])
                    dve.tensor_tensor(out=ee9[:], in0=ee9[:],
                                      in1=bcast0(L[:], KN), op=op.mult)
                    nc.sync.dma_start(out=outd[:, :, zc:zc + SZ, :],
                                      in_=ee9[:])

    nc.compile()
    return nc


# --------------------------------------------------------------------------
# Host side
# --------------------------------------------------------------------------

_CACHED = {}


def _get_nc(ks_value):
    key = float(ks_value)
    if key not in _CACHED:
        _CACHED[key] = build_bass(key)
    return _CACHED[key]


def _shard_inputs(x):
    """x: [H, M, N] f32 -> list of per-core input maps."""
    maps = []
    zext = np.arange(-1, H + 1) % H
    xs = np.arange(N)
    cstv = np.tile((1.0 + 2.0 * np.arange(27) * 2.0**-23)
                   .astype(np.float32).reshape(1, 27, 1), (128, 1, 1))
    for c in range(NCORES):
        ys = (np.arange(YS * c - 2, YS * c + YS + 2)) % M
        slab = x[zext][:, ys, :]                       # [66, 20, 128]
        a = np.empty((128, 3, ZE, YI), dtype=np.float32)
        for r in range(3):
            xrot = (xs + r - 1) % N
            a[:, r] = slab[:, :, xrot].transpose(2, 0, 1)
        maps.append({"xin": np.ascontiguousarray(a), "cst": cstv})
    return maps


def kernel(input, ksigma, k, w):
    from concourse.bass_utils import run_bass_kernel_spmd

    x = np.asarray(input, dtype=np.float32)
    assert x.shape == (H, M, N)
    ks = float(np.asarray(ksigma).reshape(-1)[0])
    assert int(k) == KN and int(w) == 3

    nc = _get_nc(ks)
    in_maps = _shard_inputs(x)
    res = run_bass_kernel_spmd(nc, in_maps, core_ids=list(range(NCORES)))
    full = np.empty((H, M, N, KN), dtype=np.float32)
    for c in range(NCORES):
        oc = res.results[c]["out"]          # [128, KN, H, YS]
        full[:, YS * c:YS * c + YS] = oc.transpose(2, 3, 0, 1)
    return full.reshape(H * M * N, KN)
